# revision 56
# baseline (speedup 1.0000x reference)
"""Trainium2 Bass kernel for EditOuterAttention (dense transformer cross-attention).

Reference computation (BS=2, LX=LY=2048, D=1024, H=16, DK=64):
    q = x @ Wq + bq ; k = y @ Wk + bk ; v = y @ Wv + bv     (per batch)
    scores = q @ k^T / sqrt(DK) + mask
    out = (softmax(scores) @ v) @ Wo + bo
Sharding: 8 cores = 2 (batch) x 4 (head groups of 4 heads / 256 dims);
host sums the 4 tensor-parallel partial O^T outputs per batch.

Schedule: the middle phase is paced by the scalar-engine exp
(~(N+310)/1.2 ns per activation).  Heads are processed in PAIRS whose
K/Q slices live at SBUF partitions 0-63 (even head) and 64-127 (odd
head): their score matmuls auto-derive PE tile_position (0,0)/(64,0)
and run CONCURRENTLY in the top/bottom halves of the PE array (row
tiling).  Score psum alternates S,P,S,P,...,S between two tensors --
SC1 [128,1024] for single-group exps and SC2 [128,2048] for PAIRED
exps (one N=2048 activation amortizes the fixed ACT overhead; 5 pairs
+ 6 singles per block vs 16 singles) -- so consecutive activations
never read the same tensor and each group's psum write-after-read
clears two exps ahead.  Exp'd scores land in per-TRIPLET SBUF tiles
(tile-level RAW then resolves per-triplet, letting the last block's
AV start during its own exps).  AV for the previous block runs head
A in windows 1-4 / head B in 5-10 (chains early), norms at w7/w12,
and the O-projection is emitted post-block (2 tiles) + carried into
the next block's first windows (2), executing during the boundary
exps; pool-rotation-sensitive tiles are pre-allocated so they pair
with already-cast slots.  Startup: weights are host-pre-swizzled to
the SBUF layout (one 4KB-line DMA each), xT/yT interleave across the
sync+scalar hardware queues xT-first, and the K-projection's
psum->SBUF copies are split scalar/vector and partially deferred so
the first exp fires right after K's last matmul.  V-projection uses
four half-bank psum groups and spreads over blocks 0-1.
"""

import numpy as np
import ml_dtypes

import concourse.bass as bass
import concourse.bacc as bacc
import concourse.tile as tile
import concourse.mybir as mybir
from concourse.bass_utils import run_bass_kernel_spmd

BS, LX, LY, D, H, DK = 2, 2048, 2048, 1024, 16, 64
NCORES = 8
NGRP = 4             # head groups (tensor-parallel)
HD = H * DK // NGRP  # 256 head dims per core
NH = H // NGRP       # 4 heads per core
SXB = 512            # sx block
NSXB = LX // SXB     # 4
NSYT = LY // 128     # 16 sy tiles
NDC = D // 128       # 8 contraction chunks
NET = D // 128       # 8 output-feature tiles

F32 = mybir.dt.float32
BF16 = mybir.dt.bfloat16
EXPF = mybir.ActivationFunctionType.Exp

_compiled = {}


def _build(has_qk_bias: bool, has_mask: bool, n_cores: int = NCORES,
           with_collective: bool = False):
    nc = bacc.Bacc("TRN2", target_bir_lowering=False, debug=False,
                   num_devices=n_cores)

    xT = nc.dram_tensor("xT", [D, LX], BF16, kind="ExternalInput")
    yT = nc.dram_tensor("yT", [D, LY], BF16, kind="ExternalInput")
    # weights arrive pre-swizzled to the SBUF layout (row p = all
    # contraction chunks for partition p) so each is ONE DMA with 4KB
    # contiguous per-partition lines -- 512B-descriptor transfers run at
    # ~90 GB/s and were blocking the activation stream on their queue
    wq = nc.dram_tensor("wq", [128, NDC * HD], BF16, kind="ExternalInput")
    wk = nc.dram_tensor("wk", [128, NDC * HD], BF16, kind="ExternalInput")
    wv = nc.dram_tensor("wv", [128, NDC * HD], BF16, kind="ExternalInput")
    wo = nc.dram_tensor("wo", [128, 2 * D], BF16, kind="ExternalInput")
    if has_qk_bias:
        bq = nc.dram_tensor("bq", [HD], F32, kind="ExternalInput")
        bk = nc.dram_tensor("bk", [HD], F32, kind="ExternalInput")
    if has_mask:
        em = nc.dram_tensor("em", [LY, LX], BF16, kind="ExternalInput")
    # bf16 partials: halves the output DMA; the host accumulates the four
    # tensor-parallel partials in fp32
    out_ext = nc.dram_tensor("out", [D, LX], BF16, kind="ExternalOutput")

    stb = 2                       # ST2 ring (32KB/partition each)
    smb = 3 if has_mask else 4    # small-chain rings
    with tile.TileContext(nc) as tc:
        with (
            tc.tile_pool(name="persist", bufs=1) as pp,
            tc.tile_pool(name="st", bufs=stb) as stp,
            tc.tile_pool(name="ostage", bufs=3) as osp,
            tc.tile_pool(name="small", bufs=3) as smp,
            # scores region: 6 PSUM banks = 3 rotating groups of [128,1024]
            tc.tile_pool(name="scp", bufs=1, space="PSUM") as scp,
            # AV accumulators / V-proj / O-proj column tiles: 2 banks
            tc.tile_pool(name="avp", bufs=2, space="PSUM") as avp,
        ):
            # ---- static inputs -> SBUF --------------------------------
            # xT streams on the sync queue, yT concurrently on the vector
            # queue (HBM is the shared limit, but one queue alone tops out
            # ~280 GB/s on descriptor issue); wq/wk ride the scalar queue,
            # wv/wo the gpsimd queue so the projections are never
            # queue-gated.
            wq_sb = pp.tile([128, NDC * HD], BF16, tag="wq")
            wk_sb = pp.tile([128, NDC * HD], BF16, tag="wk")
            wv_sb = pp.tile([128, NDC * HD], BF16, tag="wv")
            xT_sb = pp.tile([128, NDC * LX], BF16, tag="xT")
            yT_sb = pp.tile([128, NDC * LY], BF16, tag="yT")
            wo_sb = pp.tile([128, 2 * D], BF16, tag="wo")
            # xT/yT interleave across both hardware DMA queues (sync and
            # scalar; one queue alone caps ~280 GB/s, two reach ~310+).
            # xT first so the Q projection drains the PE before K's
            # matmuls queue up; wq/wk lead the scalar queue, wv/wo ride
            # the (software-DGE) gpsimd queue since they're not urgent.
            nc.sync.dma_start(out=wq_sb[:], in_=wq[:, :])
            nc.sync.dma_start(out=wk_sb[:], in_=wk[:, :])
            for d in range(NDC):
                eng = nc.scalar if d % 2 == 0 else nc.sync
                eng.dma_start(out=xT_sb[:, d * LX:(d + 1) * LX],
                              in_=xT[d * 128:(d + 1) * 128, :])
            for d in range(NDC):
                eng = nc.scalar if d % 2 == 0 else nc.sync
                eng.dma_start(out=yT_sb[:, d * LY:(d + 1) * LY],
                              in_=yT[d * 128:(d + 1) * 128, :])
            nc.sync.dma_start(out=wv_sb[:], in_=wv[:, :])
            nc.gpsimd.dma_start(out=wo_sb[:], in_=wo[:, :])
            if has_qk_bias:
                bq_sb = pp.tile([128, 2], F32, tag="bq")
                bk_sb = pp.tile([128, 2], F32, tag="bk")
                nc.scalar.dma_start(out=bq_sb[:], in_=bq.ap().rearrange("(t p) -> p t", p=128))
                nc.scalar.dma_start(out=bk_sb[:], in_=bk.ap().rearrange("(t p) -> p t", p=128))

            ones_bf = pp.tile([1, 64], BF16, tag="ones")
            nc.vector.memset(ones_bf[:], 1.0)

            # scores psum: groups k%3 in {0,1} rotate through the halves
            # of SC2 (exp'd together, one N=2048 activation), k%3==2 goes
            # to SC1 (single N=1024 activation).  Two tensors so the
            # tile-level write-after-read dependencies coincide exactly
            # with the true pipeline hazards.
            SC2 = scp.tile([128, 2048], F32, tag="sc2")
            SC1 = scp.tile([128, 1024], F32, tag="sc1")

            def sc_group(k):
                # alternating S,P,S,P,...,S: singles (SC1) at k%3==0,
                # pairs (SC2) at k%3 in {1,2}.  Consecutive exps never
                # touch the same psum tensor, so the write-after-read for
                # each group resolves two exps ahead of its own exp.
                if k % 3 == 0:
                    return SC1[:]
                return SC2[:, (k % 3 - 1) * 1024:(k % 3) * 1024]

            # ---- Q^T / K^T projections: out [hd, seq] -----------------
            QT_sb = pp.tile([128, 2 * LX], BF16, tag="QT")
            KT_sb = pp.tile([128, 2 * LY], BF16, tag="KT")

            qk_parts = [(wq_sb, xT_sb, QT_sb, "bq"),
                        (wk_sb, yT_sb, KT_sb, "bk")]

            def emit_qk_proj_paced(part, av_tiles, defer_copies=False):
                # contraction-outer so each input chunk is consumed by 8
                # matmuls as soon as its DMA lands; 8 psum groups = 6 SC
                # half-slots + the 2 AV banks.  With defer_copies the
                # psum->SBUF copies are returned as closures so the first
                # score groups can start before all of K^T is copied out.
                w_sb, src_sb, dst_sb, bias_name = qk_parts[part]

                def group_ap(g):    # g = 0..7: t0 sb0-3, then t1 sb0-3
                    if g < 4:
                        return SC2[:, g * SXB:(g + 1) * SXB]
                    if g < 6:
                        return SC1[:, (g - 4) * SXB:(g - 3) * SXB]
                    return av_tiles[g - 6][:]
                for d in range(NDC):
                    for g in range(8):
                        t, sb = (0, g) if g < 4 else (1, g - 4)
                        nc.tensor.matmul(
                            group_ap(g),
                            lhsT=w_sb[:, d * HD + t * 128: d * HD + (t + 1) * 128],
                            rhs=src_sb[:, d * LX + sb * SXB: d * LX + sb * SXB + SXB],
                            start=(d == 0), stop=(d == NDC - 1),
                            skip_group_check=True)

                def mk_copy(g):
                    def f():
                        t, sb = (0, g) if g < 4 else (1, g - 4)
                        dst = dst_sb[:, t * LX + sb * SXB: t * LX + sb * SXB + SXB]
                        if has_qk_bias:
                            b_sb = bq_sb if bias_name == "bq" else bk_sb
                            nc.vector.tensor_scalar_add(dst, group_ap(g),
                                                        b_sb[:, t:t + 1])
                        elif defer_copies and g % 2 == 0:
                            # scalar engine is idle pre-first-exp; Copy is
                            # in every ACT table set (no reload)
                            nc.scalar.copy(dst, group_ap(g))
                        else:
                            nc.vector.tensor_copy(dst, group_ap(g))
                    return f
                copies = [mk_copy(g) for g in range(8)]
                if defer_copies:
                    return copies
                for f in copies:
                    f()

            # ---- V projection: out [seq, hd] interleaved with ones ----
            # V1 layout per sy tile: [128, NH*65] = 4 x (64 v-dims + ones)
            # (padded 64 cols so AV stationaries can be read 128 wide)
            V1_sb = pp.tile([128, NSYT * NH * 65 + 64], BF16, tag="V1")

            vp_ps = {}

            def emit_v_proj(st):
                # two [128, 256] psum half-bank groups per AV-pool tile:
                # 4 outstanding V-proj groups, so the matmuls never wait
                # on the psum->SBUF copy chain
                if st // 2 not in vp_ps:
                    vp_ps[st // 2] = avp.tile([128, SXB], F32, tag="av",
                                              name=f"vp{st}")
                ps = vp_ps[st // 2][:, (st % 2) * HD:(st % 2) * HD + HD]
                for d in range(NDC):
                    nc.tensor.matmul(
                        ps,
                        lhsT=yT_sb[:, d * LY + st * 128: d * LY + st * 128 + 128],
                        rhs=wv_sb[:, d * HD:(d + 1) * HD],
                        start=(d == 0), stop=(d == NDC - 1),
                        skip_group_check=True)
                dst = V1_sb[:, st * NH * 65:(st + 1) * NH * 65] \
                    .rearrange("p (h c) -> p h c", c=65)[:, :, 0:64]
                nc.vector.tensor_copy(dst, ps.rearrange("p (h c) -> p h c", c=64))

            # ---- mask (rare path): exp(mask)^T blocks per sx block ----
            em_blocks = {}

            def load_mask_block(sb):
                mb = stp.tile([128, NSYT * SXB], BF16, tag="mask", bufs=2)
                for st in range(NSYT):
                    nc.sync.dma_start(
                        out=mb[:, st * SXB:(st + 1) * SXB],
                        in_=em[st * 128:(st + 1) * 128, sb * SXB:(sb + 1) * SXB])
                em_blocks[sb] = mb

            # ---- attention: pair-blocks (sx block, head pair) ---------
            # pair-block p = (sb, hp): heads 2hp (partitions 0-63) and
            # 2hp+1 (partitions 64-127), both reading the ht=hp column
            # half of QT/KT.  16 score groups k=0..15: head k%2, sy tiles
            # (2*(k//2), 2*(k//2)+1), psum slot k%3.  ST2 columns follow
            # k, so the paired exp of groups (k, k+1) (k%3==0) writes one
            # contiguous [*, 2048] range.
            AO_sb = pp.tile([128, 2 * LX], BF16, tag="AO")
            pblocks = [(sb, hp) for sb in range(NSXB) for hp in range(2)]
            st_tiles = {}

            def emit_pair_scores(p, fillers):
                # fillers: list of 16 callables, one run after each score
                # group's matmuls + (possibly deferred) exp
                sb, hp = pblocks[p]
                if has_mask and hp == 0:
                    load_mask_block(sb)
                # exp'd scores: one tile per triplet of groups so the AV
                # matmuls' tile-level read deps resolve as soon as that
                # triplet's exps have run (not the whole block's)
                STt = [stp.tile([128, 3072 if t < 5 else 1024], BF16,
                                tag=f"st{t}", bufs=2, name=f"st{p}_{t}")
                       for t in range(6)]
                st_tiles[p] = STt

                def st_ap(kk, lo, hi):
                    return STt[kk // 3][:, (kk % 3) * 1024 + lo:
                                        (kk % 3) * 1024 + hi]
                for k in range(16):
                    hb, g = k % 2, k // 2
                    hr = hb * 64
                    ps = sc_group(k)
                    for j in range(2):
                        st = 2 * g + j
                        nc.tensor.matmul(
                            ps[:, j * SXB:(j + 1) * SXB],
                            lhsT=KT_sb[hr:hr + 64, hp * LY + st * 128: hp * LY + st * 128 + 128],
                            rhs=QT_sb[hr:hr + 64, hp * LX + sb * SXB: hp * LX + sb * SXB + SXB],
                            start=True, stop=True,
                            skip_group_check=True)
                    if k % 3 == 0:      # single exp from SC1
                        nc.scalar.activation(st_ap(k, 0, 1024), SC1[:],
                                             EXPF, scale=1.0 / (DK ** 0.5))
                    elif k % 3 == 2:    # paired exp over both SC2 halves
                        nc.scalar.activation(st_ap(k - 1, 0, 2048), SC2[:],
                                             EXPF, scale=1.0 / (DK ** 0.5))
                    if has_mask and k % 3 != 1:
                        mb = em_blocks[sb]
                        lo = k - 1 if (k % 3 == 2) else k
                        for kk in range(lo, k + 1):
                            gg = kk // 2
                            nc.vector.tensor_mul(
                                st_ap(kk, 0, 1024), st_ap(kk, 0, 1024),
                                mb[:, gg * 1024:(gg + 1) * 1024])
                    fillers[k]()

            # ---- AV + normalize chain ---------------------------------
            norm_state = {}
            norm_rr = {}

            def emit_av_chain(key, pav, last):
                dcp = smp.tile([1, SXB], F32, tag="den", bufs=2,
                               name=f"den{key[0]}_{key[1]}")
                nc.vector.tensor_copy(dcp[:], pav[64:65, :])
                rF = smp.tile([1, SXB], F32, tag="rf", bufs=2,
                              name=f"rf{key[0]}_{key[1]}")
                # ~18-bit accurate, 5x faster than InstReciprocal; must
                # read SBUF (custom-DVE op misreads PSUM on HW)
                nc.vector.reciprocal_approx_fast(rF[:], dcp[:])
                if last:
                    rrB = smp.tile([1, SXB], BF16, tag="rr", bufs=2,
                                   name=f"rr{key[0]}_{key[1]}")
                    nc.vector.tensor_copy(rrB[:], rF[:])
                    bc = None
                    norm_rr[key] = rrB
                else:
                    bc = smp.tile([64, SXB], F32, tag="bc", bufs=3,
                                  name=f"bc{key[0]}_{key[1]}")
                    nc.gpsimd.partition_broadcast(bc[:], rF[:])
                un = smp.tile([64, SXB], BF16, tag="un", bufs=3,
                              name=f"un{key[0]}_{key[1]}")
                nc.vector.tensor_copy(un[:], pav[0:64, :])
                norm_state[key] = (un, bc)

            def av_step(p, hb, st, pav, last):
                # one sy-tile step of AV accumulation for one head of
                # pair-block p; the chain rides the head's last step
                sb, hp = pblocks[p]
                h = 2 * hp + hb
                kk = 2 * (st // 2) + hb
                STt = st_tiles[p]
                src = STt[kk // 3][:, (kk % 3) * 1024 + (st % 2) * SXB:
                                   (kk % 3) * 1024 + (st % 2) * SXB + SXB]
                # stationary read 128 wide (overreads the neighbouring
                # head's data into psum rows 65-127, which nothing reads)
                # so the weight load qualifies for FWL and stays hidden
                nc.tensor.matmul(
                    pav[:],
                    lhsT=V1_sb[:, st * NH * 65 + h * 65:
                               st * NH * 65 + h * 65 + 128],
                    rhs=src,
                    start=(st == 0), stop=(st == NSYT - 1),
                    skip_group_check=True)
                if st == NSYT - 1:
                    emit_av_chain((p, hb), pav, last)

            def emit_norm_apply(p, hb):
                sb, hp = pblocks[p]
                h = 2 * hp + hb
                hr = hb * 64
                un, bc = norm_state.pop((p, hb))
                if bc is None:
                    pbc = avp.tile([128, SXB], F32, tag="av",
                                   name=f"pbc{p}_{hb}")
                    nc.tensor.matmul(pbc[0:64, :], lhsT=ones_bf[:],
                                     rhs=norm_rr.pop((p, hb))[:],
                                     start=True, stop=True)
                    bcap = pbc[0:64, :]
                else:
                    bcap = bc[:]
                nc.vector.tensor_mul(
                    AO_sb[hr:hr + 64,
                          hp * LX + sb * SXB: hp * LX + sb * SXB + SXB],
                    un[:], bcap)

            # ---- O-projection: et tiles in the AV banks ---------------
            ost4 = {}

            def emit_oproj_et(sb, et, tail=False, po=None):
                if po is None:
                    po = avp.tile([128, SXB], F32, tag="av",
                                  name=f"po{sb}_{et}")
                for c in range(2):
                    nc.tensor.matmul(
                        po[:],
                        lhsT=wo_sb[:, c * D + et * 128: c * D + (et + 1) * 128],
                        rhs=AO_sb[:, c * LX + sb * SXB: c * LX + sb * SXB + SXB],
                        start=(c == 0), stop=(c == 1),
                        skip_group_check=True)
                half = et // 4
                if (sb, half) not in ost4:
                    ost4[(sb, half)] = osp.tile([128, 4 * SXB], BF16,
                                                tag="ost", bufs=2,
                                                name=f"ost{sb}_{half}")
                dst = out_ext.ap().rearrange(
                    "(h e2 e p) (s c) -> h s p e2 e c", h=2, e2=2, e=2,
                    s=NSXB)
                ot = ost4[(sb, half)]
                q = et % 4
                if tail and et % 2 == 0:
                    # post-last-exp the scalar engine is idle: alternate
                    # the staging casts so DVE isn't the serial resource
                    nc.scalar.copy(ot[:, q * SXB:(q + 1) * SXB], po[:])
                else:
                    nc.vector.tensor_copy(ot[:, q * SXB:(q + 1) * SXB], po[:])
                last = (sb == NSXB - 1 and half == 1)
                if last and q == 1:
                    nc.sync.dma_start(
                        out=dst[half, sb][:, 0],
                        in_=ot[:, 0:2 * SXB].rearrange("p (e c) -> p e c",
                                                       e=2))
                elif q == 3:
                    del ost4[(sb, half)]
                    if last:
                        nc.sync.dma_start(
                            out=dst[half, sb][:, 1],
                            in_=ot[:, 2 * SXB:].rearrange("p (e c) -> p e c",
                                                          e=2))
                    else:
                        nc.sync.dma_start(
                            out=dst[half, sb].rearrange("p e2 e c -> p (e2 e) c"),
                            in_=ot[:].rearrange("p (e c) -> p e c", e=4))

            # ---- emission plan ----------------------------------------
            # pair-block 0: V projection rides the windows (AV banks are
            # idle); pair-block p>0: AV(p-1) front-loaded in windows
            # 0..7, chains at window 8-ish, norms at 11/12, pending
            # O-projection et tiles in the tail windows 12..15.
            av0 = avp.tile([128, SXB], F32, tag="av", name="qk0")
            av1 = avp.tile([128, SXB], F32, tag="av", name="qk1")
            emit_qk_proj_paced(0, (av0, av1))
            av2 = avp.tile([128, SXB], F32, tag="av", name="qk2")
            av3 = avp.tile([128, SXB], F32, tag="av", name="qk3")
            k_copies = emit_qk_proj_paced(1, (av2, av3), defer_copies=True)
            ones_cols = V1_sb[:, 0:NSYT * NH * 65].rearrange(
                "p (t h c) -> p t h c", t=NSYT, c=65)[:, :, :, 64:65]
            nc.vector.memset(ones_cols, 1.0)
            nc.vector.memset(V1_sb[:, NSYT * NH * 65:], 0.0)

            def noop():
                pass

            def combine(*fs):
                def f():
                    for g in fs:
                        g()
                return f

            # pair-block 0's first score groups reuse the SC psum the K
            # projection just filled: those six copies run up front (split
            # scalar/vector), only the two AV-bank copies are deferred.
            # V projection: one tile per window from window 2 on.
            # block 0 carries only 12 V-proj tiles (1/window from w3) --
            # 16 would oversubscribe the PE vs the exp budget; the last 4
            # ride block 1's first windows (AV(0) is delayed to w3+ there)
            for g in range(6):
                k_copies[g]()
            b0_fillers = [combine(*([k_copies[6], k_copies[7]]
                                    if w == 0 else []),
                                  *([lambda v=w - 3: emit_v_proj(v)]
                                    if 3 <= w < 15 else []))
                          for w in range(16)]
            emit_pair_scores(0, b0_fillers)

            # AV(p-1) steps per window of block p: head A (fast chain) in
            # windows 1-4, head B in 5-10; norms once each broadcast has
            # landed.  O-projection tiles are emitted AFTER the block's
            # last exp so they never sit between score matmuls in the PE
            # queue -- they execute during the boundary exps instead.
            # Block 1 runs everything ~2 windows later: its first windows
            # host block 0's last V-proj tiles.
            AV_A = [(), (0, 1, 2, 3), (4, 5, 6, 7), (8, 9, 10, 11),
                    (12, 13, 14, 15)] + [()] * 11
            AV_B = [()] * 5 + [(0, 1, 2), (3, 4, 5), (6, 7, 8),
                               (9, 10, 11), (12, 13), (14, 15)] + [()] * 5
            AV_A1 = [(), (), (), (0, 1, 2, 3), (4, 5, 6, 7), (8, 9, 10, 11),
                     (12, 13, 14, 15)] + [()] * 9
            AV_B1 = [()] * 7 + [(0, 1, 2), (3, 4, 5), (6, 7, 8),
                                (9, 10, 11), (12, 13), (14, 15)] + [()] * 3

            npb = len(pblocks)
            pending_oproj = []      # (sb, et) waiting for a slot
            carry_oproj = []        # ets deferred into the next block's
                                    # first windows (execute during its
                                    # first exps, after its k0 matmuls)
            for p in range(1, npb):
                # carry-over O-projection tiles allocate BEFORE the AV
                # accumulators so pool rotation pairs them with the
                # previous boundary's (already-cast) tiles
                carries = []
                for _ in range(min(2, len(carry_oproj))):
                    sb_, et_ = carry_oproj.pop(0)
                    pot = avp.tile([128, SXB], F32, tag="av",
                                   name=f"poc{sb_}_{et_}")
                    carries.append((sb_, et_, pot))
                if p == 1:
                    # pre-allocate the carried V-proj psum tiles so pool
                    # rotation pairs them with block 0's (already-copied)
                    # V-proj tiles rather than the live AV accumulators
                    vp_ps[6] = avp.tile([128, SXB], F32, tag="av",
                                        name="vp12")
                    vp_ps[7] = avp.tile([128, SXB], F32, tag="av",
                                        name="vp14")
                pavA = avp.tile([128, SXB], F32, tag="av", name=f"avA{p-1}")
                pavB = avp.tile([128, SXB], F32, tag="av", name=f"avB{p-1}")
                avA, avB = (AV_A1, AV_B1) if p == 1 else (AV_A, AV_B)
                fillers = []
                for w in range(16):
                    fs = []
                    if w < len(carries):
                        sb_, et_, pot = carries[w]
                        fs.append(lambda s=sb_, e=et_, t=pot:
                                  emit_oproj_et(s, e, po=t))
                    if p == 1 and w < 4:
                        fs.append(lambda v=12 + w: emit_v_proj(v))
                    fs += [(lambda s=s: av_step(p - 1, 0, s, pavA, False))
                           for s in avA[w]]
                    fs += [(lambda s=s: av_step(p - 1, 1, s, pavB, False))
                           for s in avB[w]]
                    fillers.append(combine(*fs))
                # norms for p-1 once the gpsimd broadcast has landed
                nwA, nwB = (9, 14) if p == 1 else (7, 12)
                fillers[nwA] = combine(fillers[nwA],
                                       lambda p_=p - 1: emit_norm_apply(p_, 0))
                fillers[nwB] = combine(fillers[nwB],
                                       lambda p_=p - 1: emit_norm_apply(p_, 1))
                if p == npb - 1:
                    # pre-emit the last block's own AV for already-exp'd
                    # triplets into its tail windows
                    pav7A = avp.tile([128, SXB], F32, tag="av", name="avAL")
                    pav7B = avp.tile([128, SXB], F32, tag="av", name="avBL")
                    pre = [(0, s) for s in range(9)] + [(1, s) for s in range(9)]
                    for w in range(10, 16):
                        take, pre = pre[:3], pre[3:]
                        fillers[w] = combine(fillers[w], *[
                            (lambda hb=hb, s=s: av_step(
                                npb - 1, hb, s, pav7A if hb == 0 else pav7B,
                                False))
                            for hb, s in take])
                emit_pair_scores(p, fillers)
                # O-projection of the sx block whose norms were applied
                # during THIS block's windows (emitted post-block, so the
                # matmuls execute during the boundary exps)
                osb, hpdone = pblocks[p - 1]
                if hpdone == 1:
                    pending_oproj.extend((osb, et) for et in range(NET))
                if p < npb - 1:
                    for _ in range(2):
                        if pending_oproj:
                            sb_, et_ = pending_oproj.pop(0)
                            emit_oproj_et(sb_, et_)
                    for _ in range(2):
                        if pending_oproj:
                            carry_oproj.append(pending_oproj.pop(0))

            # tail: remaining AV steps of the last pair-block; leftover
            # O-projection tiles fill the PE while the final normalize
            # chains run on DVE/gpsimd.  The last sx block's O-projection
            # uses the (now dead) scores psum banks as 8 dedicated
            # half-bank groups: its c=0 contraction half (hp=0 heads,
            # normed at window 7) runs during the final exps, and c=1
            # lands after the last norms with no pool-rotation stalls.
            o3_av = {}

            def oproj3_group(et):
                if et < 4:
                    return SC2[:, et * SXB:(et + 1) * SXB]
                if et < 6:
                    return SC1[:, (et - 4) * SXB:(et - 3) * SXB]
                if et not in o3_av:
                    o3_av[et] = avp.tile([128, SXB], F32, tag="av",
                                         name=f"o3_{et}")
                return o3_av[et][:]

            def oproj3_mm(et, c):
                nc.tensor.matmul(
                    oproj3_group(et),
                    lhsT=wo_sb[:, c * D + et * 128: c * D + (et + 1) * 128],
                    rhs=AO_sb[:, c * LX + (NSXB - 1) * SXB:
                              c * LX + (NSXB - 1) * SXB + SXB],
                    start=(c == 0), stop=(c == 1),
                    skip_group_check=True)

            for et in range(4):     # SC2 groups free after the last pair
                oproj3_mm(et, 0)
            for st in range(9, NSYT):
                av_step(npb - 1, 0, st, pav7A, False)
                av_step(npb - 1, 1, st, pav7B, st == NSYT - 1)
            for et in range(4, 6):     # SC1 frees after the last exp
                oproj3_mm(et, 0)
            while pending_oproj:
                sb_, et_ = pending_oproj.pop(0)
                emit_oproj_et(sb_, et_, tail=True)
            emit_norm_apply(npb - 1, 0)
            emit_norm_apply(npb - 1, 1)
            ot3 = [osp.tile([128, 4 * SXB], BF16, tag="ost", bufs=2,
                            name=f"ot3_{half}") for half in range(2)]
            dst3 = out_ext.ap().rearrange(
                "(h e2 e p) (s c) -> h s p e2 e c", h=2, e2=2, e=2, s=NSXB)
            for et in range(NET):
                if et >= 6:
                    oproj3_mm(et, 0)
                oproj3_mm(et, 1)
                if et % 2 == 0:
                    nc.scalar.copy(ot3[et // 4][:, (et % 4) * SXB:
                                                (et % 4 + 1) * SXB],
                                   oproj3_group(et))
                else:
                    nc.vector.tensor_copy(ot3[et // 4][:, (et % 4) * SXB:
                                                       (et % 4 + 1) * SXB],
                                          oproj3_group(et))
                if et == 3:
                    nc.sync.dma_start(
                        out=dst3[0, NSXB - 1].rearrange(
                            "p e2 e c -> p (e2 e) c"),
                        in_=ot3[0][:].rearrange("p (e c) -> p e c", e=4))
            nc.sync.dma_start(
                out=dst3[1, NSXB - 1][:, 0],
                in_=ot3[1][:, 0:2 * SXB].rearrange("p (e c) -> p e c", e=2))
            nc.sync.dma_start(
                out=dst3[1, NSXB - 1][:, 1],
                in_=ot3[1][:, 2 * SXB:].rearrange("p (e c) -> p e c", e=2))

    nc.compile()
    return nc


def _get_compiled(has_qk_bias: bool, has_mask: bool):
    key = (has_qk_bias, has_mask)
    if key not in _compiled:
        _compiled[key] = _build(has_qk_bias, has_mask)
    return _compiled[key]


def _prep_inputs(x, y, mask, Wq, bq, Wk, bk, Wv, bv, Wo, bo,
                 has_qk_bias, has_mask):
    bf = ml_dtypes.bfloat16
    xT = [np.ascontiguousarray(x[b].T).astype(bf) for b in range(BS)]
    yT = [np.ascontiguousarray(y[b].T).astype(bf) for b in range(BS)]
    if has_mask:
        em = [np.ascontiguousarray(np.exp(mask[b, 0]).T).astype(bf)
              for b in range(BS)]
    def swz(W):
        # [n*128, m] -> [128, n*m]: row p holds all contraction chunks
        # for partition p (matches the SBUF tile layout; 4KB DMA lines)
        n = W.shape[0] // 128
        return np.ascontiguousarray(
            W.reshape(n, 128, -1).transpose(1, 0, 2).reshape(128, -1)
        ).astype(bf)

    in_maps = []
    for c in range(NCORES):
        b, g = c // NGRP, c % NGRP
        sl = slice(g * HD, (g + 1) * HD)
        m = {
            "xT": xT[b], "yT": yT[b],
            "wq": swz(Wq[:, sl]),
            "wk": swz(Wk[:, sl]),
            "wv": swz(Wv[:, sl]),
            "wo": swz(Wo[sl, :]),
        }
        if has_qk_bias:
            m["bq"] = np.ascontiguousarray(bq[sl]).astype(np.float32)
            m["bk"] = np.ascontiguousarray(bk[sl]).astype(np.float32)
        if has_mask:
            m["em"] = em[b]
        in_maps.append(m)
    return in_maps


def kernel(x, y, mask, Wq, bq, Wk, bk, Wv, bv, Wo, bo):
    x = np.asarray(x, np.float32)
    y = np.asarray(y, np.float32)
    mask = np.asarray(mask, np.float32)
    has_qk_bias = bool(np.any(bq) or np.any(bk))
    has_mask = bool(np.any(mask))
    nc = _get_compiled(has_qk_bias, has_mask)
    in_maps = _prep_inputs(x, y, mask, Wq, bq, Wk, bk, Wv, bv, Wo, bo,
                           has_qk_bias, has_mask)
    res = run_bass_kernel_spmd(nc, in_maps, list(range(NCORES)))
    out = np.empty((BS, LX, D), np.float32)
    for b in range(BS):
        OT = res.results[b * NGRP]["out"].astype(np.float32)
        for r in range(1, NGRP):
            OT += res.results[b * NGRP + r]["out"].astype(np.float32)
        out[b] = OT.T
    bv = np.asarray(bv, np.float32)
    bo = np.asarray(bo, np.float32)
    if bv.any() or bo.any():
        # softmax rows sum to 1 => v-bias passes through attention exactly
        out += (bv @ np.asarray(Wo, np.float32) + bo)[None, None, :]
    return out


# revision 57
# speedup vs baseline: 1.0075x; 1.0075x over previous
"""Trainium2 Bass kernel for EditOuterAttention (dense transformer cross-attention).

Reference computation (BS=2, LX=LY=2048, D=1024, H=16, DK=64):
    q = x @ Wq + bq ; k = y @ Wk + bk ; v = y @ Wv + bv     (per batch)
    scores = q @ k^T / sqrt(DK) + mask
    out = (softmax(scores) @ v) @ Wo + bo
Sharding: 8 cores = 2 (batch) x 4 (head groups of 4 heads / 256 dims);
host sums the 4 tensor-parallel partial O^T outputs per batch.

Schedule: the middle phase is paced by the scalar-engine exp
(~(N+310)/1.2 ns per activation).  Heads are processed in PAIRS whose
K/Q slices live at SBUF partitions 0-63 (even head) and 64-127 (odd
head): their score matmuls auto-derive PE tile_position (0,0)/(64,0)
and run CONCURRENTLY in the top/bottom halves of the PE array (row
tiling).  Score psum alternates S,P,S,P,...,S between two tensors --
SC1 [128,1024] for single-group exps and SC2 [128,2048] for PAIRED
exps (one N=2048 activation amortizes the fixed ACT overhead; 5 pairs
+ 6 singles per block vs 16 singles) -- so consecutive activations
never read the same tensor and each group's psum write-after-read
clears two exps ahead.  Exp'd scores land in per-TRIPLET SBUF tiles
(tile-level RAW then resolves per-triplet, letting the last block's
AV start during its own exps).  AV for the previous block runs head
A in windows 1-4 / head B in 5-10 (chains early), norms at w7/w12,
and the O-projection is emitted post-block (2 tiles) + carried into
the next block's first windows (2), executing during the boundary
exps; pool-rotation-sensitive tiles are pre-allocated so they pair
with already-cast slots.  Startup: weights are host-pre-swizzled to
the SBUF layout (one 4KB-line DMA each), xT/yT interleave across the
sync+scalar hardware queues xT-first, and the K-projection's
psum->SBUF copies are split scalar/vector and partially deferred so
the first exp fires right after K's last matmul.  V-projection uses
four half-bank psum groups and spreads over blocks 0-1.
"""

import numpy as np
import ml_dtypes

import concourse.bass as bass
import concourse.bacc as bacc
import concourse.tile as tile
import concourse.mybir as mybir
from concourse.bass_utils import run_bass_kernel_spmd

BS, LX, LY, D, H, DK = 2, 2048, 2048, 1024, 16, 64
NCORES = 8
NGRP = 4             # head groups (tensor-parallel)
HD = H * DK // NGRP  # 256 head dims per core
NH = H // NGRP       # 4 heads per core
SXB = 512            # sx block
NSXB = LX // SXB     # 4
NSYT = LY // 128     # 16 sy tiles
NDC = D // 128       # 8 contraction chunks
NET = D // 128       # 8 output-feature tiles

F32 = mybir.dt.float32
BF16 = mybir.dt.bfloat16
EXPF = mybir.ActivationFunctionType.Exp

_compiled = {}


def _build(has_qk_bias: bool, has_mask: bool, n_cores: int = NCORES,
           with_collective: bool = False):
    nc = bacc.Bacc("TRN2", target_bir_lowering=False, debug=False,
                   num_devices=n_cores)

    xT = nc.dram_tensor("xT", [D, LX], BF16, kind="ExternalInput")
    yT = nc.dram_tensor("yT", [D, LY], BF16, kind="ExternalInput")
    # weights arrive pre-swizzled to the SBUF layout (row p = all
    # contraction chunks for partition p) so each is ONE DMA with 4KB
    # contiguous per-partition lines -- 512B-descriptor transfers run at
    # ~90 GB/s and were blocking the activation stream on their queue
    wq = nc.dram_tensor("wq", [128, NDC * HD], BF16, kind="ExternalInput")
    wk = nc.dram_tensor("wk", [128, NDC * HD], BF16, kind="ExternalInput")
    wv = nc.dram_tensor("wv", [128, NDC * HD], BF16, kind="ExternalInput")
    wo = nc.dram_tensor("wo", [128, 2 * D], BF16, kind="ExternalInput")
    if has_qk_bias:
        bq = nc.dram_tensor("bq", [HD], F32, kind="ExternalInput")
        bk = nc.dram_tensor("bk", [HD], F32, kind="ExternalInput")
    if has_mask:
        em = nc.dram_tensor("em", [LY, LX], BF16, kind="ExternalInput")
    # bf16 partials: halves the output DMA; the host accumulates the four
    # tensor-parallel partials in fp32
    out_ext = nc.dram_tensor("out", [D, LX], BF16, kind="ExternalOutput")

    stb = 2                       # ST2 ring (32KB/partition each)
    smb = 3 if has_mask else 4    # small-chain rings
    with tile.TileContext(nc) as tc:
        with (
            tc.tile_pool(name="persist", bufs=1) as pp,
            tc.tile_pool(name="st", bufs=stb) as stp,
            tc.tile_pool(name="ostage", bufs=3) as osp,
            tc.tile_pool(name="small", bufs=3) as smp,
            # scores region: 6 PSUM banks = 3 rotating groups of [128,1024]
            tc.tile_pool(name="scp", bufs=1, space="PSUM") as scp,
            # AV accumulators / V-proj / O-proj column tiles: 2 banks
            tc.tile_pool(name="avp", bufs=2, space="PSUM") as avp,
        ):
            # ---- static inputs -> SBUF --------------------------------
            # xT streams on the sync queue, yT concurrently on the vector
            # queue (HBM is the shared limit, but one queue alone tops out
            # ~280 GB/s on descriptor issue); wq/wk ride the scalar queue,
            # wv/wo the gpsimd queue so the projections are never
            # queue-gated.
            wq_sb = pp.tile([128, NDC * HD], BF16, tag="wq")
            wk_sb = pp.tile([128, NDC * HD], BF16, tag="wk")
            wv_sb = pp.tile([128, NDC * HD], BF16, tag="wv")
            xT_sb = pp.tile([128, NDC * LX], BF16, tag="xT")
            yT_sb = pp.tile([128, NDC * LY], BF16, tag="yT")
            wo_sb = pp.tile([128, 2 * D], BF16, tag="wo")
            # xT/yT interleave across both hardware DMA queues (sync and
            # scalar; one queue alone caps ~280 GB/s, two reach ~310+).
            # xT first so the Q projection drains the PE before K's
            # matmuls queue up; wq/wk lead the scalar queue, wv/wo ride
            # the (software-DGE) gpsimd queue since they're not urgent.
            nc.sync.dma_start(out=wq_sb[:], in_=wq[:, :])
            nc.sync.dma_start(out=wk_sb[:], in_=wk[:, :])
            for d in range(NDC):
                eng = nc.scalar if d % 2 == 0 else nc.sync
                eng.dma_start(out=xT_sb[:, d * LX:(d + 1) * LX],
                              in_=xT[d * 128:(d + 1) * 128, :])
            for d in range(NDC):
                eng = nc.scalar if d % 2 == 0 else nc.sync
                eng.dma_start(out=yT_sb[:, d * LY:(d + 1) * LY],
                              in_=yT[d * 128:(d + 1) * 128, :])
            nc.sync.dma_start(out=wv_sb[:], in_=wv[:, :])
            nc.gpsimd.dma_start(out=wo_sb[:], in_=wo[:, :])
            if has_qk_bias:
                bq_sb = pp.tile([128, 2], F32, tag="bq")
                bk_sb = pp.tile([128, 2], F32, tag="bk")
                nc.scalar.dma_start(out=bq_sb[:], in_=bq.ap().rearrange("(t p) -> p t", p=128))
                nc.scalar.dma_start(out=bk_sb[:], in_=bk.ap().rearrange("(t p) -> p t", p=128))

            ones_bf = pp.tile([1, 64], BF16, tag="ones")
            nc.vector.memset(ones_bf[:], 1.0)

            # scores psum: groups k%3 in {0,1} rotate through the halves
            # of SC2 (exp'd together, one N=2048 activation), k%3==2 goes
            # to SC1 (single N=1024 activation).  Two tensors so the
            # tile-level write-after-read dependencies coincide exactly
            # with the true pipeline hazards.
            SC2 = scp.tile([128, 2048], F32, tag="sc2")
            SC1 = scp.tile([128, 1024], F32, tag="sc1")

            def sc_group(k):
                # alternating S,P,S,P,...,S: singles (SC1) at k%3==0,
                # pairs (SC2) at k%3 in {1,2}.  Consecutive exps never
                # touch the same psum tensor, so the write-after-read for
                # each group resolves two exps ahead of its own exp.
                if k % 3 == 0:
                    return SC1[:]
                return SC2[:, (k % 3 - 1) * 1024:(k % 3) * 1024]

            # ---- Q^T / K^T projections: out [hd, seq] -----------------
            QT_sb = pp.tile([128, 2 * LX], BF16, tag="QT")
            KT_sb = pp.tile([128, 2 * LY], BF16, tag="KT")

            qk_parts = [(wq_sb, xT_sb, QT_sb, "bq"),
                        (wk_sb, yT_sb, KT_sb, "bk")]

            def emit_qk_proj_paced(part, av_tiles, defer_copies=False):
                # contraction-outer so each input chunk is consumed by 8
                # matmuls as soon as its DMA lands; 8 psum groups = 6 SC
                # half-slots + the 2 AV banks.  With defer_copies the
                # psum->SBUF copies are returned as closures so the first
                # score groups can start before all of K^T is copied out.
                w_sb, src_sb, dst_sb, bias_name = qk_parts[part]

                def group_ap(g):    # g = 0..7: t0 sb0-3, then t1 sb0-3
                    if g < 4:
                        return SC2[:, g * SXB:(g + 1) * SXB]
                    if g < 6:
                        return SC1[:, (g - 4) * SXB:(g - 3) * SXB]
                    return av_tiles[g - 6][:]
                for d in range(NDC):
                    for g in range(8):
                        t, sb = (0, g) if g < 4 else (1, g - 4)
                        nc.tensor.matmul(
                            group_ap(g),
                            lhsT=w_sb[:, d * HD + t * 128: d * HD + (t + 1) * 128],
                            rhs=src_sb[:, d * LX + sb * SXB: d * LX + sb * SXB + SXB],
                            start=(d == 0), stop=(d == NDC - 1),
                            skip_group_check=True)

                def mk_copy(g):
                    def f():
                        t, sb = (0, g) if g < 4 else (1, g - 4)
                        dst = dst_sb[:, t * LX + sb * SXB: t * LX + sb * SXB + SXB]
                        if has_qk_bias:
                            b_sb = bq_sb if bias_name == "bq" else bk_sb
                            nc.vector.tensor_scalar_add(dst, group_ap(g),
                                                        b_sb[:, t:t + 1])
                        elif defer_copies and g % 2 == 0:
                            # scalar engine is idle pre-first-exp; Copy is
                            # in every ACT table set (no reload)
                            nc.scalar.copy(dst, group_ap(g))
                        else:
                            nc.vector.tensor_copy(dst, group_ap(g))
                    return f
                copies = [mk_copy(g) for g in range(8)]
                if defer_copies:
                    return copies
                for f in copies:
                    f()

            # ---- V projection: out [seq, hd] interleaved with ones ----
            # V1 layout per sy tile: [128, NH*65] = 4 x (64 v-dims + ones)
            # (padded 64 cols so AV stationaries can be read 128 wide)
            V1_sb = pp.tile([128, NSYT * NH * 65 + 64], BF16, tag="V1")

            vp_ps = {}

            def emit_v_proj(st):
                # two [128, 256] psum half-bank groups per AV-pool tile:
                # 4 outstanding V-proj groups, so the matmuls never wait
                # on the psum->SBUF copy chain
                if st // 2 not in vp_ps:
                    vp_ps[st // 2] = avp.tile([128, SXB], F32, tag="av",
                                              name=f"vp{st}")
                ps = vp_ps[st // 2][:, (st % 2) * HD:(st % 2) * HD + HD]
                for d in range(NDC):
                    nc.tensor.matmul(
                        ps,
                        lhsT=yT_sb[:, d * LY + st * 128: d * LY + st * 128 + 128],
                        rhs=wv_sb[:, d * HD:(d + 1) * HD],
                        start=(d == 0), stop=(d == NDC - 1),
                        skip_group_check=True)
                dst = V1_sb[:, st * NH * 65:(st + 1) * NH * 65] \
                    .rearrange("p (h c) -> p h c", c=65)[:, :, 0:64]
                nc.vector.tensor_copy(dst, ps.rearrange("p (h c) -> p h c", c=64))

            # ---- mask (rare path): exp(mask)^T blocks per sx block ----
            em_blocks = {}

            def load_mask_block(sb):
                mb = stp.tile([128, NSYT * SXB], BF16, tag="mask", bufs=2)
                for st in range(NSYT):
                    nc.sync.dma_start(
                        out=mb[:, st * SXB:(st + 1) * SXB],
                        in_=em[st * 128:(st + 1) * 128, sb * SXB:(sb + 1) * SXB])
                em_blocks[sb] = mb

            # ---- attention: pair-blocks (sx block, head pair) ---------
            # pair-block p = (sb, hp): heads 2hp (partitions 0-63) and
            # 2hp+1 (partitions 64-127), both reading the ht=hp column
            # half of QT/KT.  16 score groups k=0..15: head k%2, sy tiles
            # (2*(k//2), 2*(k//2)+1), psum slot k%3.  ST2 columns follow
            # k, so the paired exp of groups (k, k+1) (k%3==0) writes one
            # contiguous [*, 2048] range.
            AO_sb = pp.tile([128, 2 * LX], BF16, tag="AO")
            pblocks = [(sb, hp) for sb in range(NSXB) for hp in range(2)]
            st_tiles = {}

            def emit_pair_scores(p, fillers):
                # fillers: list of 16 callables, one run after each score
                # group's matmuls + (possibly deferred) exp
                sb, hp = pblocks[p]
                if has_mask and hp == 0:
                    load_mask_block(sb)
                # exp'd scores: one tile per triplet of groups so the AV
                # matmuls' tile-level read deps resolve as soon as that
                # triplet's exps have run (not the whole block's)
                STt = [stp.tile([128, 3072 if t < 5 else 1024], BF16,
                                tag=f"st{t}", bufs=2, name=f"st{p}_{t}")
                       for t in range(6)]
                st_tiles[p] = STt

                def st_ap(kk, lo, hi):
                    return STt[kk // 3][:, (kk % 3) * 1024 + lo:
                                        (kk % 3) * 1024 + hi]
                for k in range(16):
                    hb, g = k % 2, k // 2
                    hr = hb * 64
                    ps = sc_group(k)
                    for j in range(2):
                        st = 2 * g + j
                        nc.tensor.matmul(
                            ps[:, j * SXB:(j + 1) * SXB],
                            lhsT=KT_sb[hr:hr + 64, hp * LY + st * 128: hp * LY + st * 128 + 128],
                            rhs=QT_sb[hr:hr + 64, hp * LX + sb * SXB: hp * LX + sb * SXB + SXB],
                            start=True, stop=True,
                            skip_group_check=True)
                    if k % 3 == 0:      # single exp from SC1
                        nc.scalar.activation(st_ap(k, 0, 1024), SC1[:],
                                             EXPF, scale=1.0 / (DK ** 0.5))
                    elif k % 3 == 2:    # paired exp over both SC2 halves
                        nc.scalar.activation(st_ap(k - 1, 0, 2048), SC2[:],
                                             EXPF, scale=1.0 / (DK ** 0.5))
                    if has_mask and k % 3 != 1:
                        mb = em_blocks[sb]
                        lo = k - 1 if (k % 3 == 2) else k
                        for kk in range(lo, k + 1):
                            gg = kk // 2
                            nc.vector.tensor_mul(
                                st_ap(kk, 0, 1024), st_ap(kk, 0, 1024),
                                mb[:, gg * 1024:(gg + 1) * 1024])
                    fillers[k]()

            # ---- AV + normalize chain ---------------------------------
            norm_state = {}
            norm_rr = {}

            def emit_av_chain(key, pav, last):
                dcp = smp.tile([1, SXB], F32, tag="den", bufs=2,
                               name=f"den{key[0]}_{key[1]}")
                nc.vector.tensor_copy(dcp[:], pav[64:65, :])
                rF = smp.tile([1, SXB], F32, tag="rf", bufs=2,
                              name=f"rf{key[0]}_{key[1]}")
                # ~18-bit accurate, 5x faster than InstReciprocal; must
                # read SBUF (custom-DVE op misreads PSUM on HW)
                nc.vector.reciprocal_approx_fast(rF[:], dcp[:])
                if last:
                    rrB = smp.tile([1, SXB], BF16, tag="rr", bufs=2,
                                   name=f"rr{key[0]}_{key[1]}")
                    nc.vector.tensor_copy(rrB[:], rF[:])
                    bc = None
                    norm_rr[key] = rrB
                else:
                    bc = smp.tile([64, SXB], F32, tag="bc", bufs=3,
                                  name=f"bc{key[0]}_{key[1]}")
                    nc.gpsimd.partition_broadcast(bc[:], rF[:])
                un = smp.tile([64, SXB], BF16, tag="un", bufs=3,
                              name=f"un{key[0]}_{key[1]}")
                nc.vector.tensor_copy(un[:], pav[0:64, :])
                norm_state[key] = (un, bc)

            def av_step(p, hb, st, pav, last):
                # one sy-tile step of AV accumulation for one head of
                # pair-block p; the chain rides the head's last step
                sb, hp = pblocks[p]
                h = 2 * hp + hb
                kk = 2 * (st // 2) + hb
                STt = st_tiles[p]
                src = STt[kk // 3][:, (kk % 3) * 1024 + (st % 2) * SXB:
                                   (kk % 3) * 1024 + (st % 2) * SXB + SXB]
                # stationary read 128 wide (overreads the neighbouring
                # head's data into psum rows 65-127, which nothing reads)
                # so the weight load qualifies for FWL and stays hidden
                nc.tensor.matmul(
                    pav[:],
                    lhsT=V1_sb[:, st * NH * 65 + h * 65:
                               st * NH * 65 + h * 65 + 128],
                    rhs=src,
                    start=(st == 0), stop=(st == NSYT - 1),
                    skip_group_check=True)
                if st == NSYT - 1:
                    emit_av_chain((p, hb), pav, last)

            def emit_norm_apply(p, hb):
                sb, hp = pblocks[p]
                h = 2 * hp + hb
                hr = hb * 64
                un, bc = norm_state.pop((p, hb))
                if bc is None:
                    pbc = avp.tile([128, SXB], F32, tag="av",
                                   name=f"pbc{p}_{hb}")
                    nc.tensor.matmul(pbc[0:64, :], lhsT=ones_bf[:],
                                     rhs=norm_rr.pop((p, hb))[:],
                                     start=True, stop=True)
                    bcap = pbc[0:64, :]
                else:
                    bcap = bc[:]
                nc.vector.tensor_mul(
                    AO_sb[hr:hr + 64,
                          hp * LX + sb * SXB: hp * LX + sb * SXB + SXB],
                    un[:], bcap)

            # ---- O-projection: et tiles in the AV banks ---------------
            ost4 = {}

            def emit_oproj_et(sb, et, tail=False, po=None):
                if po is None:
                    po = avp.tile([128, SXB], F32, tag="av",
                                  name=f"po{sb}_{et}")
                for c in range(2):
                    nc.tensor.matmul(
                        po[:],
                        lhsT=wo_sb[:, c * D + et * 128: c * D + (et + 1) * 128],
                        rhs=AO_sb[:, c * LX + sb * SXB: c * LX + sb * SXB + SXB],
                        start=(c == 0), stop=(c == 1),
                        skip_group_check=True)
                half = et // 4
                if (sb, half) not in ost4:
                    ost4[(sb, half)] = osp.tile([128, 4 * SXB], BF16,
                                                tag="ost", bufs=2,
                                                name=f"ost{sb}_{half}")
                dst = out_ext.ap().rearrange(
                    "(h e2 e p) (s c) -> h s p e2 e c", h=2, e2=2, e=2,
                    s=NSXB)
                ot = ost4[(sb, half)]
                q = et % 4
                if tail and et % 2 == 0:
                    # post-last-exp the scalar engine is idle: alternate
                    # the staging casts so DVE isn't the serial resource
                    nc.scalar.copy(ot[:, q * SXB:(q + 1) * SXB], po[:])
                else:
                    nc.vector.tensor_copy(ot[:, q * SXB:(q + 1) * SXB], po[:])
                last = (sb == NSXB - 1 and half == 1)
                if last and q == 1:
                    nc.sync.dma_start(
                        out=dst[half, sb][:, 0],
                        in_=ot[:, 0:2 * SXB].rearrange("p (e c) -> p e c",
                                                       e=2))
                elif q == 3:
                    del ost4[(sb, half)]
                    if last:
                        nc.sync.dma_start(
                            out=dst[half, sb][:, 1],
                            in_=ot[:, 2 * SXB:].rearrange("p (e c) -> p e c",
                                                          e=2))
                    else:
                        nc.sync.dma_start(
                            out=dst[half, sb].rearrange("p e2 e c -> p (e2 e) c"),
                            in_=ot[:].rearrange("p (e c) -> p e c", e=4))

            # ---- emission plan ----------------------------------------
            # pair-block 0: V projection rides the windows (AV banks are
            # idle); pair-block p>0: AV(p-1) front-loaded in windows
            # 0..7, chains at window 8-ish, norms at 11/12, pending
            # O-projection et tiles in the tail windows 12..15.
            av0 = avp.tile([128, SXB], F32, tag="av", name="qk0")
            av1 = avp.tile([128, SXB], F32, tag="av", name="qk1")
            emit_qk_proj_paced(0, (av0, av1))
            av2 = avp.tile([128, SXB], F32, tag="av", name="qk2")
            av3 = avp.tile([128, SXB], F32, tag="av", name="qk3")
            k_copies = emit_qk_proj_paced(1, (av2, av3), defer_copies=True)
            ones_cols = V1_sb[:, 0:NSYT * NH * 65].rearrange(
                "p (t h c) -> p t h c", t=NSYT, c=65)[:, :, :, 64:65]
            nc.vector.memset(ones_cols, 1.0)
            nc.vector.memset(V1_sb[:, NSYT * NH * 65:], 0.0)

            def noop():
                pass

            def combine(*fs):
                def f():
                    for g in fs:
                        g()
                return f

            # pair-block 0's first score groups reuse the SC psum the K
            # projection just filled: those six copies run up front (split
            # scalar/vector), only the two AV-bank copies are deferred.
            # V projection: one tile per window from window 2 on.
            # block 0 carries only 12 V-proj tiles (1/window from w3) --
            # 16 would oversubscribe the PE vs the exp budget; the last 4
            # ride block 1's first windows (AV(0) is delayed to w3+ there)
            for g in range(6):
                k_copies[g]()
            b0_fillers = [combine(*([k_copies[6], k_copies[7]]
                                    if w == 0 else []),
                                  *([lambda v=w - 3: emit_v_proj(v)]
                                    if 3 <= w < 15 else []))
                          for w in range(16)]
            emit_pair_scores(0, b0_fillers)

            # AV(p-1) steps per window of block p: head A (fast chain) in
            # windows 1-4, head B in 5-10; norms once each broadcast has
            # landed.  O-projection tiles are emitted AFTER the block's
            # last exp so they never sit between score matmuls in the PE
            # queue -- they execute during the boundary exps instead.
            # Block 1 runs everything ~2 windows later: its first windows
            # host block 0's last V-proj tiles.
            AV_A = [(), (0, 1, 2, 3), (4, 5, 6, 7), (8, 9, 10, 11),
                    (12, 13, 14, 15)] + [()] * 11
            AV_B = [()] * 5 + [(0, 1, 2), (3, 4, 5), (6, 7, 8),
                               (9, 10, 11), (12, 13), (14, 15)] + [()] * 5
            AV_A1 = [(), (), (), (0, 1, 2, 3), (4, 5, 6, 7), (8, 9, 10, 11),
                     (12, 13, 14, 15)] + [()] * 9
            AV_B1 = [()] * 7 + [(0, 1, 2), (3, 4, 5), (6, 7, 8),
                                (9, 10, 11), (12, 13), (14, 15)] + [()] * 3

            npb = len(pblocks)
            pending_oproj = []      # (sb, et) waiting for a slot
            carry_oproj = []        # ets deferred into the next block's
                                    # first windows (execute during its
                                    # first exps, after its k0 matmuls)
            for p in range(1, npb):
                # carry-over O-projection tiles allocate BEFORE the AV
                # accumulators so pool rotation pairs them with the
                # previous boundary's (already-cast) tiles
                carries = []
                for _ in range(min(2, len(carry_oproj))):
                    sb_, et_ = carry_oproj.pop(0)
                    pot = avp.tile([128, SXB], F32, tag="av",
                                   name=f"poc{sb_}_{et_}")
                    carries.append((sb_, et_, pot))
                if p == 1:
                    # pre-allocate the carried V-proj psum tiles so pool
                    # rotation pairs them with block 0's (already-copied)
                    # V-proj tiles rather than the live AV accumulators
                    vp_ps[6] = avp.tile([128, SXB], F32, tag="av",
                                        name="vp12")
                    vp_ps[7] = avp.tile([128, SXB], F32, tag="av",
                                        name="vp14")
                pavA = avp.tile([128, SXB], F32, tag="av", name=f"avA{p-1}")
                pavB = avp.tile([128, SXB], F32, tag="av", name=f"avB{p-1}")
                avA, avB = (AV_A1, AV_B1) if p == 1 else (AV_A, AV_B)
                fillers = []
                for w in range(16):
                    fs = []
                    if w < len(carries):
                        sb_, et_, pot = carries[w]
                        fs.append(lambda s=sb_, e=et_, t=pot:
                                  emit_oproj_et(s, e, po=t))
                    if p == 1 and w < 4:
                        fs.append(lambda v=12 + w: emit_v_proj(v))
                    fs += [(lambda s=s: av_step(p - 1, 0, s, pavA, False))
                           for s in avA[w]]
                    fs += [(lambda s=s: av_step(p - 1, 1, s, pavB, False))
                           for s in avB[w]]
                    fillers.append(combine(*fs))
                # norms for p-1 once the gpsimd broadcast has landed
                nwA, nwB = (9, 14) if p == 1 else (7, 12)
                fillers[nwA] = combine(fillers[nwA],
                                       lambda p_=p - 1: emit_norm_apply(p_, 0))
                fillers[nwB] = combine(fillers[nwB],
                                       lambda p_=p - 1: emit_norm_apply(p_, 1))
                if p == npb - 1:
                    # pre-emit the last block's own AV for already-exp'd
                    # triplets into its tail windows
                    pav7A = avp.tile([128, SXB], F32, tag="av", name="avAL")
                    pav7B = avp.tile([128, SXB], F32, tag="av", name="avBL")
                    pre = [(0, s) for s in range(9)] + [(1, s) for s in range(9)]
                    for w in range(10, 16):
                        take, pre = pre[:3], pre[3:]
                        fillers[w] = combine(fillers[w], *[
                            (lambda hb=hb, s=s: av_step(
                                npb - 1, hb, s, pav7A if hb == 0 else pav7B,
                                False))
                            for hb, s in take])
                emit_pair_scores(p, fillers)
                # O-projection of the sx block whose norms were applied
                # during THIS block's windows (emitted post-block, so the
                # matmuls execute during the boundary exps)
                osb, hpdone = pblocks[p - 1]
                if hpdone == 1:
                    pending_oproj.extend((osb, et) for et in range(NET))
                if p < npb - 1:
                    for _ in range(2):
                        if pending_oproj:
                            sb_, et_ = pending_oproj.pop(0)
                            emit_oproj_et(sb_, et_)
                    for _ in range(2):
                        if pending_oproj:
                            carry_oproj.append(pending_oproj.pop(0))

            # tail: remaining AV steps of the last pair-block; leftover
            # O-projection tiles fill the PE while the final normalize
            # chains run on DVE/gpsimd; then the last sx block's
            # O-projection
            for st in range(9, NSYT):
                av_step(npb - 1, 0, st, pav7A, False)
                av_step(npb - 1, 1, st, pav7B, st == NSYT - 1)
            while pending_oproj:
                sb_, et_ = pending_oproj.pop(0)
                emit_oproj_et(sb_, et_, tail=True)
            emit_norm_apply(npb - 1, 0)
            emit_norm_apply(npb - 1, 1)
            for et in range(NET):
                emit_oproj_et(NSXB - 1, et, tail=True)

    nc.compile()
    return nc


def _get_compiled(has_qk_bias: bool, has_mask: bool):
    key = (has_qk_bias, has_mask)
    if key not in _compiled:
        _compiled[key] = _build(has_qk_bias, has_mask)
    return _compiled[key]


def _prep_inputs(x, y, mask, Wq, bq, Wk, bk, Wv, bv, Wo, bo,
                 has_qk_bias, has_mask):
    bf = ml_dtypes.bfloat16
    xT = [np.ascontiguousarray(x[b].T).astype(bf) for b in range(BS)]
    yT = [np.ascontiguousarray(y[b].T).astype(bf) for b in range(BS)]
    if has_mask:
        em = [np.ascontiguousarray(np.exp(mask[b, 0]).T).astype(bf)
              for b in range(BS)]
    def swz(W):
        # [n*128, m] -> [128, n*m]: row p holds all contraction chunks
        # for partition p (matches the SBUF tile layout; 4KB DMA lines)
        n = W.shape[0] // 128
        return np.ascontiguousarray(
            W.reshape(n, 128, -1).transpose(1, 0, 2).reshape(128, -1)
        ).astype(bf)

    in_maps = []
    for c in range(NCORES):
        b, g = c // NGRP, c % NGRP
        sl = slice(g * HD, (g + 1) * HD)
        m = {
            "xT": xT[b], "yT": yT[b],
            "wq": swz(Wq[:, sl]),
            "wk": swz(Wk[:, sl]),
            "wv": swz(Wv[:, sl]),
            "wo": swz(Wo[sl, :]),
        }
        if has_qk_bias:
            m["bq"] = np.ascontiguousarray(bq[sl]).astype(np.float32)
            m["bk"] = np.ascontiguousarray(bk[sl]).astype(np.float32)
        if has_mask:
            m["em"] = em[b]
        in_maps.append(m)
    return in_maps


def kernel(x, y, mask, Wq, bq, Wk, bk, Wv, bv, Wo, bo):
    x = np.asarray(x, np.float32)
    y = np.asarray(y, np.float32)
    mask = np.asarray(mask, np.float32)
    has_qk_bias = bool(np.any(bq) or np.any(bk))
    has_mask = bool(np.any(mask))
    nc = _get_compiled(has_qk_bias, has_mask)
    in_maps = _prep_inputs(x, y, mask, Wq, bq, Wk, bk, Wv, bv, Wo, bo,
                           has_qk_bias, has_mask)
    res = run_bass_kernel_spmd(nc, in_maps, list(range(NCORES)))
    out = np.empty((BS, LX, D), np.float32)
    for b in range(BS):
        OT = res.results[b * NGRP]["out"].astype(np.float32)
        for r in range(1, NGRP):
            OT += res.results[b * NGRP + r]["out"].astype(np.float32)
        out[b] = OT.T
    bv = np.asarray(bv, np.float32)
    bo = np.asarray(bo, np.float32)
    if bv.any() or bo.any():
        # softmax rows sum to 1 => v-bias passes through attention exactly
        out += (bv @ np.asarray(Wo, np.float32) + bo)[None, None, :]
    return out


# revision 58
# speedup vs baseline: 1.0083x; 1.0008x over previous
"""Trainium2 Bass kernel for EditOuterAttention (dense transformer cross-attention).

Reference computation (BS=2, LX=LY=2048, D=1024, H=16, DK=64):
    q = x @ Wq + bq ; k = y @ Wk + bk ; v = y @ Wv + bv     (per batch)
    scores = q @ k^T / sqrt(DK) + mask
    out = (softmax(scores) @ v) @ Wo + bo
Sharding: 8 cores = 2 (batch) x 4 (head groups of 4 heads / 256 dims);
host sums the 4 tensor-parallel partial O^T outputs per batch.

Schedule: the middle phase is paced by the scalar-engine exp
(~(N+310)/1.2 ns per activation).  Heads are processed in PAIRS whose
K/Q slices live at SBUF partitions 0-63 (even head) and 64-127 (odd
head): their score matmuls auto-derive PE tile_position (0,0)/(64,0)
and run CONCURRENTLY in the top/bottom halves of the PE array (row
tiling).  Score psum alternates S,P,S,P,...,S between two tensors --
SC1 [128,1024] for single-group exps and SC2 [128,2048] for PAIRED
exps (one N=2048 activation amortizes the fixed ACT overhead; 5 pairs
+ 6 singles per block vs 16 singles) -- so consecutive activations
never read the same tensor and each group's psum write-after-read
clears two exps ahead.  Exp'd scores land in per-TRIPLET SBUF tiles
(tile-level RAW then resolves per-triplet, letting the last block's
AV start during its own exps).  AV for the previous block runs head
A in windows 1-4 / head B in 5-10 (chains early), norms at w7/w12,
and the O-projection is emitted post-block (2 tiles) + carried into
the next block's first windows (2), executing during the boundary
exps; pool-rotation-sensitive tiles are pre-allocated so they pair
with already-cast slots.  Startup: weights are host-pre-swizzled to
the SBUF layout (one 4KB-line DMA each), xT/yT interleave across the
sync+scalar hardware queues xT-first, and the K-projection's
psum->SBUF copies are split scalar/vector and partially deferred so
the first exp fires right after K's last matmul.  V-projection uses
four half-bank psum groups and spreads over blocks 0-1.
"""

import numpy as np
import ml_dtypes

import concourse.bass as bass
import concourse.bacc as bacc
import concourse.tile as tile
import concourse.mybir as mybir
from concourse.bass_utils import run_bass_kernel_spmd

BS, LX, LY, D, H, DK = 2, 2048, 2048, 1024, 16, 64
NCORES = 8
NGRP = 4             # head groups (tensor-parallel)
HD = H * DK // NGRP  # 256 head dims per core
NH = H // NGRP       # 4 heads per core
SXB = 512            # sx block
NSXB = LX // SXB     # 4
NSYT = LY // 128     # 16 sy tiles
NDC = D // 128       # 8 contraction chunks
NET = D // 128       # 8 output-feature tiles

F32 = mybir.dt.float32
BF16 = mybir.dt.bfloat16
EXPF = mybir.ActivationFunctionType.Exp

_compiled = {}


def _build(has_qk_bias: bool, has_mask: bool, n_cores: int = NCORES,
           with_collective: bool = False):
    nc = bacc.Bacc("TRN2", target_bir_lowering=False, debug=False,
                   num_devices=n_cores)

    xT = nc.dram_tensor("xT", [D, LX], BF16, kind="ExternalInput")
    yT = nc.dram_tensor("yT", [D, LY], BF16, kind="ExternalInput")
    # weights arrive pre-swizzled to the SBUF layout (row p = all
    # contraction chunks for partition p) so each is ONE DMA with 4KB
    # contiguous per-partition lines -- 512B-descriptor transfers run at
    # ~90 GB/s and were blocking the activation stream on their queue
    wq = nc.dram_tensor("wq", [128, NDC * HD], BF16, kind="ExternalInput")
    wk = nc.dram_tensor("wk", [128, NDC * HD], BF16, kind="ExternalInput")
    wv = nc.dram_tensor("wv", [128, NDC * HD], BF16, kind="ExternalInput")
    wo = nc.dram_tensor("wo", [128, 2 * D], BF16, kind="ExternalInput")
    if has_qk_bias:
        bq = nc.dram_tensor("bq", [HD], F32, kind="ExternalInput")
        bk = nc.dram_tensor("bk", [HD], F32, kind="ExternalInput")
    if has_mask:
        em = nc.dram_tensor("em", [LY, LX], BF16, kind="ExternalInput")
    # bf16 partials: halves the output DMA; the host accumulates the four
    # tensor-parallel partials in fp32
    out_ext = nc.dram_tensor("out", [D, LX], BF16, kind="ExternalOutput")

    stb = 2                       # ST2 ring (32KB/partition each)
    smb = 3 if has_mask else 4    # small-chain rings
    with tile.TileContext(nc) as tc:
        with (
            tc.tile_pool(name="persist", bufs=1) as pp,
            tc.tile_pool(name="st", bufs=stb) as stp,
            tc.tile_pool(name="ostage", bufs=3) as osp,
            tc.tile_pool(name="small", bufs=3) as smp,
            # scores region: 6 PSUM banks = 3 rotating groups of [128,1024]
            tc.tile_pool(name="scp", bufs=1, space="PSUM") as scp,
            # AV accumulators / V-proj / O-proj column tiles: 2 banks
            tc.tile_pool(name="avp", bufs=2, space="PSUM") as avp,
        ):
            # ---- static inputs -> SBUF --------------------------------
            # xT streams on the sync queue, yT concurrently on the vector
            # queue (HBM is the shared limit, but one queue alone tops out
            # ~280 GB/s on descriptor issue); wq/wk ride the scalar queue,
            # wv/wo the gpsimd queue so the projections are never
            # queue-gated.
            wq_sb = pp.tile([128, NDC * HD], BF16, tag="wq")
            wk_sb = pp.tile([128, NDC * HD], BF16, tag="wk")
            wv_sb = pp.tile([128, NDC * HD], BF16, tag="wv")
            xT_sb = pp.tile([128, NDC * LX], BF16, tag="xT")
            yT_sb = pp.tile([128, NDC * LY], BF16, tag="yT")
            wo_sb = pp.tile([128, 2 * D], BF16, tag="wo")
            # xT/yT interleave across both hardware DMA queues (sync and
            # scalar; one queue alone caps ~280 GB/s, two reach ~310+).
            # xT first so the Q projection drains the PE before K's
            # matmuls queue up; wq/wk lead the scalar queue, wv/wo ride
            # the (software-DGE) gpsimd queue since they're not urgent.
            nc.sync.dma_start(out=wq_sb[:], in_=wq[:, :])
            nc.sync.dma_start(out=wk_sb[:], in_=wk[:, :])
            # each chunk split in half across the two queues: a half
            # gates only 4 projection matmuls, so the DMA-paced Q/K
            # phase stalls on 2KB-line half-chunks instead of full ones
            for d in range(NDC):
                for h in range(2):
                    eng = nc.scalar if (d + h) % 2 == 0 else nc.sync
                    eng.dma_start(
                        out=xT_sb[:, d * LX + h * 1024: d * LX + h * 1024 + 1024],
                        in_=xT[d * 128:(d + 1) * 128, h * 1024:(h + 1) * 1024])
            for d in range(NDC):
                for h in range(2):
                    eng = nc.scalar if (d + h) % 2 == 0 else nc.sync
                    eng.dma_start(
                        out=yT_sb[:, d * LY + h * 1024: d * LY + h * 1024 + 1024],
                        in_=yT[d * 128:(d + 1) * 128, h * 1024:(h + 1) * 1024])
            nc.sync.dma_start(out=wv_sb[:], in_=wv[:, :])
            nc.gpsimd.dma_start(out=wo_sb[:], in_=wo[:, :])
            if has_qk_bias:
                bq_sb = pp.tile([128, 2], F32, tag="bq")
                bk_sb = pp.tile([128, 2], F32, tag="bk")
                nc.scalar.dma_start(out=bq_sb[:], in_=bq.ap().rearrange("(t p) -> p t", p=128))
                nc.scalar.dma_start(out=bk_sb[:], in_=bk.ap().rearrange("(t p) -> p t", p=128))

            ones_bf = pp.tile([1, 64], BF16, tag="ones")
            nc.vector.memset(ones_bf[:], 1.0)

            # scores psum: groups k%3 in {0,1} rotate through the halves
            # of SC2 (exp'd together, one N=2048 activation), k%3==2 goes
            # to SC1 (single N=1024 activation).  Two tensors so the
            # tile-level write-after-read dependencies coincide exactly
            # with the true pipeline hazards.
            SC2 = scp.tile([128, 2048], F32, tag="sc2")
            SC1 = scp.tile([128, 1024], F32, tag="sc1")

            def sc_group(k):
                # alternating S,P,S,P,...,S: singles (SC1) at k%3==0,
                # pairs (SC2) at k%3 in {1,2}.  Consecutive exps never
                # touch the same psum tensor, so the write-after-read for
                # each group resolves two exps ahead of its own exp.
                if k % 3 == 0:
                    return SC1[:]
                return SC2[:, (k % 3 - 1) * 1024:(k % 3) * 1024]

            # ---- Q^T / K^T projections: out [hd, seq] -----------------
            QT_sb = pp.tile([128, 2 * LX], BF16, tag="QT")
            KT_sb = pp.tile([128, 2 * LY], BF16, tag="KT")

            qk_parts = [(wq_sb, xT_sb, QT_sb, "bq"),
                        (wk_sb, yT_sb, KT_sb, "bk")]

            def emit_qk_proj_paced(part, av_tiles, defer_copies=False):
                # contraction-outer so each input chunk is consumed by 8
                # matmuls as soon as its DMA lands; 8 psum groups = 6 SC
                # half-slots + the 2 AV banks.  With defer_copies the
                # psum->SBUF copies are returned as closures so the first
                # score groups can start before all of K^T is copied out.
                w_sb, src_sb, dst_sb, bias_name = qk_parts[part]

                def group_ap(g):    # g = 0..7: t0 sb0-3, then t1 sb0-3
                    if g < 4:
                        return SC2[:, g * SXB:(g + 1) * SXB]
                    if g < 6:
                        return SC1[:, (g - 4) * SXB:(g - 3) * SXB]
                    return av_tiles[g - 6][:]
                for d in range(NDC):
                    for g in range(8):
                        t, sb = (0, g) if g < 4 else (1, g - 4)
                        nc.tensor.matmul(
                            group_ap(g),
                            lhsT=w_sb[:, d * HD + t * 128: d * HD + (t + 1) * 128],
                            rhs=src_sb[:, d * LX + sb * SXB: d * LX + sb * SXB + SXB],
                            start=(d == 0), stop=(d == NDC - 1),
                            skip_group_check=True)

                def mk_copy(g):
                    def f():
                        t, sb = (0, g) if g < 4 else (1, g - 4)
                        dst = dst_sb[:, t * LX + sb * SXB: t * LX + sb * SXB + SXB]
                        if has_qk_bias:
                            b_sb = bq_sb if bias_name == "bq" else bk_sb
                            nc.vector.tensor_scalar_add(dst, group_ap(g),
                                                        b_sb[:, t:t + 1])
                        elif defer_copies and g % 2 == 0:
                            # scalar engine is idle pre-first-exp; Copy is
                            # in every ACT table set (no reload)
                            nc.scalar.copy(dst, group_ap(g))
                        else:
                            nc.vector.tensor_copy(dst, group_ap(g))
                    return f
                copies = [mk_copy(g) for g in range(8)]
                if defer_copies:
                    return copies
                for f in copies:
                    f()

            # ---- V projection: out [seq, hd] interleaved with ones ----
            # V1 layout per sy tile: [128, NH*65] = 4 x (64 v-dims + ones)
            # (padded 64 cols so AV stationaries can be read 128 wide)
            V1_sb = pp.tile([128, NSYT * NH * 65 + 64], BF16, tag="V1")

            vp_ps = {}

            def emit_v_proj(st):
                # two [128, 256] psum half-bank groups per AV-pool tile:
                # 4 outstanding V-proj groups, so the matmuls never wait
                # on the psum->SBUF copy chain
                if st // 2 not in vp_ps:
                    vp_ps[st // 2] = avp.tile([128, SXB], F32, tag="av",
                                              name=f"vp{st}")
                ps = vp_ps[st // 2][:, (st % 2) * HD:(st % 2) * HD + HD]
                for d in range(NDC):
                    nc.tensor.matmul(
                        ps,
                        lhsT=yT_sb[:, d * LY + st * 128: d * LY + st * 128 + 128],
                        rhs=wv_sb[:, d * HD:(d + 1) * HD],
                        start=(d == 0), stop=(d == NDC - 1),
                        skip_group_check=True)
                dst = V1_sb[:, st * NH * 65:(st + 1) * NH * 65] \
                    .rearrange("p (h c) -> p h c", c=65)[:, :, 0:64]
                nc.vector.tensor_copy(dst, ps.rearrange("p (h c) -> p h c", c=64))

            # ---- mask (rare path): exp(mask)^T blocks per sx block ----
            em_blocks = {}

            def load_mask_block(sb):
                mb = stp.tile([128, NSYT * SXB], BF16, tag="mask", bufs=2)
                for st in range(NSYT):
                    nc.sync.dma_start(
                        out=mb[:, st * SXB:(st + 1) * SXB],
                        in_=em[st * 128:(st + 1) * 128, sb * SXB:(sb + 1) * SXB])
                em_blocks[sb] = mb

            # ---- attention: pair-blocks (sx block, head pair) ---------
            # pair-block p = (sb, hp): heads 2hp (partitions 0-63) and
            # 2hp+1 (partitions 64-127), both reading the ht=hp column
            # half of QT/KT.  16 score groups k=0..15: head k%2, sy tiles
            # (2*(k//2), 2*(k//2)+1), psum slot k%3.  ST2 columns follow
            # k, so the paired exp of groups (k, k+1) (k%3==0) writes one
            # contiguous [*, 2048] range.
            AO_sb = pp.tile([128, 2 * LX], BF16, tag="AO")
            pblocks = [(sb, hp) for sb in range(NSXB) for hp in range(2)]
            st_tiles = {}

            def emit_pair_scores(p, fillers):
                # fillers: list of 16 callables, one run after each score
                # group's matmuls + (possibly deferred) exp
                sb, hp = pblocks[p]
                if has_mask and hp == 0:
                    load_mask_block(sb)
                # exp'd scores: one tile per triplet of groups so the AV
                # matmuls' tile-level read deps resolve as soon as that
                # triplet's exps have run (not the whole block's)
                STt = [stp.tile([128, 3072 if t < 5 else 1024], BF16,
                                tag=f"st{t}", bufs=2, name=f"st{p}_{t}")
                       for t in range(6)]
                st_tiles[p] = STt

                def st_ap(kk, lo, hi):
                    return STt[kk // 3][:, (kk % 3) * 1024 + lo:
                                        (kk % 3) * 1024 + hi]
                for k in range(16):
                    hb, g = k % 2, k // 2
                    hr = hb * 64
                    ps = sc_group(k)
                    for j in range(2):
                        st = 2 * g + j
                        nc.tensor.matmul(
                            ps[:, j * SXB:(j + 1) * SXB],
                            lhsT=KT_sb[hr:hr + 64, hp * LY + st * 128: hp * LY + st * 128 + 128],
                            rhs=QT_sb[hr:hr + 64, hp * LX + sb * SXB: hp * LX + sb * SXB + SXB],
                            start=True, stop=True,
                            skip_group_check=True)
                    if k % 3 == 0:      # single exp from SC1
                        nc.scalar.activation(st_ap(k, 0, 1024), SC1[:],
                                             EXPF, scale=1.0 / (DK ** 0.5))
                    elif k % 3 == 2:    # paired exp over both SC2 halves
                        nc.scalar.activation(st_ap(k - 1, 0, 2048), SC2[:],
                                             EXPF, scale=1.0 / (DK ** 0.5))
                    if has_mask and k % 3 != 1:
                        mb = em_blocks[sb]
                        lo = k - 1 if (k % 3 == 2) else k
                        for kk in range(lo, k + 1):
                            gg = kk // 2
                            nc.vector.tensor_mul(
                                st_ap(kk, 0, 1024), st_ap(kk, 0, 1024),
                                mb[:, gg * 1024:(gg + 1) * 1024])
                    fillers[k]()

            # ---- AV + normalize chain ---------------------------------
            norm_state = {}
            norm_rr = {}

            def emit_av_chain(key, pav, last):
                dcp = smp.tile([1, SXB], F32, tag="den", bufs=2,
                               name=f"den{key[0]}_{key[1]}")
                nc.vector.tensor_copy(dcp[:], pav[64:65, :])
                rF = smp.tile([1, SXB], F32, tag="rf", bufs=2,
                              name=f"rf{key[0]}_{key[1]}")
                # ~18-bit accurate, 5x faster than InstReciprocal; must
                # read SBUF (custom-DVE op misreads PSUM on HW)
                nc.vector.reciprocal_approx_fast(rF[:], dcp[:])
                if last:
                    rrB = smp.tile([1, SXB], BF16, tag="rr", bufs=2,
                                   name=f"rr{key[0]}_{key[1]}")
                    nc.vector.tensor_copy(rrB[:], rF[:])
                    bc = None
                    norm_rr[key] = rrB
                else:
                    bc = smp.tile([64, SXB], F32, tag="bc", bufs=3,
                                  name=f"bc{key[0]}_{key[1]}")
                    nc.gpsimd.partition_broadcast(bc[:], rF[:])
                un = smp.tile([64, SXB], BF16, tag="un", bufs=3,
                              name=f"un{key[0]}_{key[1]}")
                nc.vector.tensor_copy(un[:], pav[0:64, :])
                norm_state[key] = (un, bc)

            def av_step(p, hb, st, pav, last):
                # one sy-tile step of AV accumulation for one head of
                # pair-block p; the chain rides the head's last step
                sb, hp = pblocks[p]
                h = 2 * hp + hb
                kk = 2 * (st // 2) + hb
                STt = st_tiles[p]
                src = STt[kk // 3][:, (kk % 3) * 1024 + (st % 2) * SXB:
                                   (kk % 3) * 1024 + (st % 2) * SXB + SXB]
                # stationary read 128 wide (overreads the neighbouring
                # head's data into psum rows 65-127, which nothing reads)
                # so the weight load qualifies for FWL and stays hidden
                nc.tensor.matmul(
                    pav[:],
                    lhsT=V1_sb[:, st * NH * 65 + h * 65:
                               st * NH * 65 + h * 65 + 128],
                    rhs=src,
                    start=(st == 0), stop=(st == NSYT - 1),
                    skip_group_check=True)
                if st == NSYT - 1:
                    emit_av_chain((p, hb), pav, last)

            def emit_norm_apply(p, hb):
                sb, hp = pblocks[p]
                h = 2 * hp + hb
                hr = hb * 64
                un, bc = norm_state.pop((p, hb))
                if bc is None:
                    pbc = avp.tile([128, SXB], F32, tag="av",
                                   name=f"pbc{p}_{hb}")
                    nc.tensor.matmul(pbc[0:64, :], lhsT=ones_bf[:],
                                     rhs=norm_rr.pop((p, hb))[:],
                                     start=True, stop=True)
                    bcap = pbc[0:64, :]
                else:
                    bcap = bc[:]
                nc.vector.tensor_mul(
                    AO_sb[hr:hr + 64,
                          hp * LX + sb * SXB: hp * LX + sb * SXB + SXB],
                    un[:], bcap)

            # ---- O-projection: et tiles in the AV banks ---------------
            ost4 = {}

            def emit_oproj_et(sb, et, tail=False, po=None):
                if po is None:
                    po = avp.tile([128, SXB], F32, tag="av",
                                  name=f"po{sb}_{et}")
                for c in range(2):
                    nc.tensor.matmul(
                        po[:],
                        lhsT=wo_sb[:, c * D + et * 128: c * D + (et + 1) * 128],
                        rhs=AO_sb[:, c * LX + sb * SXB: c * LX + sb * SXB + SXB],
                        start=(c == 0), stop=(c == 1),
                        skip_group_check=True)
                half = et // 4
                if (sb, half) not in ost4:
                    ost4[(sb, half)] = osp.tile([128, 4 * SXB], BF16,
                                                tag="ost", bufs=2,
                                                name=f"ost{sb}_{half}")
                dst = out_ext.ap().rearrange(
                    "(h e2 e p) (s c) -> h s p e2 e c", h=2, e2=2, e=2,
                    s=NSXB)
                ot = ost4[(sb, half)]
                q = et % 4
                if tail and et % 2 == 0:
                    # post-last-exp the scalar engine is idle: alternate
                    # the staging casts so DVE isn't the serial resource
                    nc.scalar.copy(ot[:, q * SXB:(q + 1) * SXB], po[:])
                else:
                    nc.vector.tensor_copy(ot[:, q * SXB:(q + 1) * SXB], po[:])
                last = (sb == NSXB - 1 and half == 1)
                if last and q == 1:
                    nc.sync.dma_start(
                        out=dst[half, sb][:, 0],
                        in_=ot[:, 0:2 * SXB].rearrange("p (e c) -> p e c",
                                                       e=2))
                elif q == 3:
                    del ost4[(sb, half)]
                    if last:
                        nc.sync.dma_start(
                            out=dst[half, sb][:, 1],
                            in_=ot[:, 2 * SXB:].rearrange("p (e c) -> p e c",
                                                          e=2))
                    else:
                        nc.sync.dma_start(
                            out=dst[half, sb].rearrange("p e2 e c -> p (e2 e) c"),
                            in_=ot[:].rearrange("p (e c) -> p e c", e=4))

            # ---- emission plan ----------------------------------------
            # pair-block 0: V projection rides the windows (AV banks are
            # idle); pair-block p>0: AV(p-1) front-loaded in windows
            # 0..7, chains at window 8-ish, norms at 11/12, pending
            # O-projection et tiles in the tail windows 12..15.
            av0 = avp.tile([128, SXB], F32, tag="av", name="qk0")
            av1 = avp.tile([128, SXB], F32, tag="av", name="qk1")
            emit_qk_proj_paced(0, (av0, av1))
            av2 = avp.tile([128, SXB], F32, tag="av", name="qk2")
            av3 = avp.tile([128, SXB], F32, tag="av", name="qk3")
            k_copies = emit_qk_proj_paced(1, (av2, av3), defer_copies=True)
            ones_cols = V1_sb[:, 0:NSYT * NH * 65].rearrange(
                "p (t h c) -> p t h c", t=NSYT, c=65)[:, :, :, 64:65]
            nc.vector.memset(ones_cols, 1.0)
            nc.vector.memset(V1_sb[:, NSYT * NH * 65:], 0.0)

            def noop():
                pass

            def combine(*fs):
                def f():
                    for g in fs:
                        g()
                return f

            # pair-block 0's first score groups reuse the SC psum the K
            # projection just filled: those six copies run up front (split
            # scalar/vector), only the two AV-bank copies are deferred.
            # V projection: one tile per window from window 2 on.
            # block 0 carries only 12 V-proj tiles (1/window from w3) --
            # 16 would oversubscribe the PE vs the exp budget; the last 4
            # ride block 1's first windows (AV(0) is delayed to w3+ there)
            for g in range(6):
                k_copies[g]()
            b0_fillers = [combine(*([k_copies[6], k_copies[7]]
                                    if w == 0 else []),
                                  *([lambda v=w - 3: emit_v_proj(v)]
                                    if 3 <= w < 15 else []))
                          for w in range(16)]
            emit_pair_scores(0, b0_fillers)

            # AV(p-1) steps per window of block p: head A (fast chain) in
            # windows 1-4, head B in 5-10; norms once each broadcast has
            # landed.  O-projection tiles are emitted AFTER the block's
            # last exp so they never sit between score matmuls in the PE
            # queue -- they execute during the boundary exps instead.
            # Block 1 runs everything ~2 windows later: its first windows
            # host block 0's last V-proj tiles.
            AV_A = [(), (0, 1, 2, 3), (4, 5, 6, 7), (8, 9, 10, 11),
                    (12, 13, 14, 15)] + [()] * 11
            AV_B = [()] * 5 + [(0, 1, 2), (3, 4, 5), (6, 7, 8),
                               (9, 10, 11), (12, 13), (14, 15)] + [()] * 5
            AV_A1 = [(), (), (), (0, 1, 2, 3), (4, 5, 6, 7), (8, 9, 10, 11),
                     (12, 13, 14, 15)] + [()] * 9
            AV_B1 = [()] * 7 + [(0, 1, 2), (3, 4, 5), (6, 7, 8),
                                (9, 10, 11), (12, 13), (14, 15)] + [()] * 3

            npb = len(pblocks)
            pending_oproj = []      # (sb, et) waiting for a slot
            carry_oproj = []        # ets deferred into the next block's
                                    # first windows (execute during its
                                    # first exps, after its k0 matmuls)
            for p in range(1, npb):
                # carry-over O-projection tiles allocate BEFORE the AV
                # accumulators so pool rotation pairs them with the
                # previous boundary's (already-cast) tiles
                carries = []
                for _ in range(min(2, len(carry_oproj))):
                    sb_, et_ = carry_oproj.pop(0)
                    pot = avp.tile([128, SXB], F32, tag="av",
                                   name=f"poc{sb_}_{et_}")
                    carries.append((sb_, et_, pot))
                if p == 1:
                    # pre-allocate the carried V-proj psum tiles so pool
                    # rotation pairs them with block 0's (already-copied)
                    # V-proj tiles rather than the live AV accumulators
                    vp_ps[6] = avp.tile([128, SXB], F32, tag="av",
                                        name="vp12")
                    vp_ps[7] = avp.tile([128, SXB], F32, tag="av",
                                        name="vp14")
                pavA = avp.tile([128, SXB], F32, tag="av", name=f"avA{p-1}")
                pavB = avp.tile([128, SXB], F32, tag="av", name=f"avB{p-1}")
                avA, avB = (AV_A1, AV_B1) if p == 1 else (AV_A, AV_B)
                fillers = []
                for w in range(16):
                    fs = []
                    if w < len(carries):
                        sb_, et_, pot = carries[w]
                        fs.append(lambda s=sb_, e=et_, t=pot:
                                  emit_oproj_et(s, e, po=t))
                    if p == 1 and w < 4:
                        fs.append(lambda v=12 + w: emit_v_proj(v))
                    fs += [(lambda s=s: av_step(p - 1, 0, s, pavA, False))
                           for s in avA[w]]
                    fs += [(lambda s=s: av_step(p - 1, 1, s, pavB, False))
                           for s in avB[w]]
                    fillers.append(combine(*fs))
                # norms for p-1 once the gpsimd broadcast has landed
                nwA, nwB = (9, 14) if p == 1 else (7, 12)
                fillers[nwA] = combine(fillers[nwA],
                                       lambda p_=p - 1: emit_norm_apply(p_, 0))
                fillers[nwB] = combine(fillers[nwB],
                                       lambda p_=p - 1: emit_norm_apply(p_, 1))
                if p == npb - 1:
                    # pre-emit the last block's own AV for already-exp'd
                    # triplets into its tail windows
                    pav7A = avp.tile([128, SXB], F32, tag="av", name="avAL")
                    pav7B = avp.tile([128, SXB], F32, tag="av", name="avBL")
                    pre = [(0, s) for s in range(9)] + [(1, s) for s in range(9)]
                    for w in range(10, 16):
                        take, pre = pre[:3], pre[3:]
                        fillers[w] = combine(fillers[w], *[
                            (lambda hb=hb, s=s: av_step(
                                npb - 1, hb, s, pav7A if hb == 0 else pav7B,
                                False))
                            for hb, s in take])
                emit_pair_scores(p, fillers)
                # O-projection of the sx block whose norms were applied
                # during THIS block's windows (emitted post-block, so the
                # matmuls execute during the boundary exps)
                osb, hpdone = pblocks[p - 1]
                if hpdone == 1:
                    pending_oproj.extend((osb, et) for et in range(NET))
                if p < npb - 1:
                    for _ in range(2):
                        if pending_oproj:
                            sb_, et_ = pending_oproj.pop(0)
                            emit_oproj_et(sb_, et_)
                    for _ in range(2):
                        if pending_oproj:
                            carry_oproj.append(pending_oproj.pop(0))

            # tail: remaining AV steps of the last pair-block; leftover
            # O-projection tiles fill the PE while the final normalize
            # chains run on DVE/gpsimd; then the last sx block's
            # O-projection
            for st in range(9, NSYT):
                av_step(npb - 1, 0, st, pav7A, False)
                av_step(npb - 1, 1, st, pav7B, st == NSYT - 1)
            while pending_oproj:
                sb_, et_ = pending_oproj.pop(0)
                emit_oproj_et(sb_, et_, tail=True)
            emit_norm_apply(npb - 1, 0)
            emit_norm_apply(npb - 1, 1)
            for et in range(NET):
                emit_oproj_et(NSXB - 1, et, tail=True)

    nc.compile()
    return nc


def _get_compiled(has_qk_bias: bool, has_mask: bool):
    key = (has_qk_bias, has_mask)
    if key not in _compiled:
        _compiled[key] = _build(has_qk_bias, has_mask)
    return _compiled[key]


def _prep_inputs(x, y, mask, Wq, bq, Wk, bk, Wv, bv, Wo, bo,
                 has_qk_bias, has_mask):
    bf = ml_dtypes.bfloat16
    xT = [np.ascontiguousarray(x[b].T).astype(bf) for b in range(BS)]
    yT = [np.ascontiguousarray(y[b].T).astype(bf) for b in range(BS)]
    if has_mask:
        em = [np.ascontiguousarray(np.exp(mask[b, 0]).T).astype(bf)
              for b in range(BS)]
    def swz(W):
        # [n*128, m] -> [128, n*m]: row p holds all contraction chunks
        # for partition p (matches the SBUF tile layout; 4KB DMA lines)
        n = W.shape[0] // 128
        return np.ascontiguousarray(
            W.reshape(n, 128, -1).transpose(1, 0, 2).reshape(128, -1)
        ).astype(bf)

    in_maps = []
    for c in range(NCORES):
        b, g = c // NGRP, c % NGRP
        sl = slice(g * HD, (g + 1) * HD)
        m = {
            "xT": xT[b], "yT": yT[b],
            "wq": swz(Wq[:, sl]),
            "wk": swz(Wk[:, sl]),
            "wv": swz(Wv[:, sl]),
            "wo": swz(Wo[sl, :]),
        }
        if has_qk_bias:
            m["bq"] = np.ascontiguousarray(bq[sl]).astype(np.float32)
            m["bk"] = np.ascontiguousarray(bk[sl]).astype(np.float32)
        if has_mask:
            m["em"] = em[b]
        in_maps.append(m)
    return in_maps


def kernel(x, y, mask, Wq, bq, Wk, bk, Wv, bv, Wo, bo):
    x = np.asarray(x, np.float32)
    y = np.asarray(y, np.float32)
    mask = np.asarray(mask, np.float32)
    has_qk_bias = bool(np.any(bq) or np.any(bk))
    has_mask = bool(np.any(mask))
    nc = _get_compiled(has_qk_bias, has_mask)
    in_maps = _prep_inputs(x, y, mask, Wq, bq, Wk, bk, Wv, bv, Wo, bo,
                           has_qk_bias, has_mask)
    res = run_bass_kernel_spmd(nc, in_maps, list(range(NCORES)))
    out = np.empty((BS, LX, D), np.float32)
    for b in range(BS):
        OT = res.results[b * NGRP]["out"].astype(np.float32)
        for r in range(1, NGRP):
            OT += res.results[b * NGRP + r]["out"].astype(np.float32)
        out[b] = OT.T
    bv = np.asarray(bv, np.float32)
    bo = np.asarray(bo, np.float32)
    if bv.any() or bo.any():
        # softmax rows sum to 1 => v-bias passes through attention exactly
        out += (bv @ np.asarray(Wo, np.float32) + bo)[None, None, :]
    return out


# revision 59
# speedup vs baseline: 1.0158x; 1.0075x over previous
"""Trainium2 Bass kernel for EditOuterAttention (dense transformer cross-attention).

Reference computation (BS=2, LX=LY=2048, D=1024, H=16, DK=64):
    q = x @ Wq + bq ; k = y @ Wk + bk ; v = y @ Wv + bv     (per batch)
    scores = q @ k^T / sqrt(DK) + mask
    out = (softmax(scores) @ v) @ Wo + bo
Sharding: 8 cores = 2 (batch) x 4 (head groups of 4 heads / 256 dims);
host sums the 4 tensor-parallel partial O^T outputs per batch.

Schedule: the middle phase is paced by the scalar-engine exp
(~(N+310)/1.2 ns per activation).  Heads are processed in PAIRS whose
K/Q slices live at SBUF partitions 0-63 (even head) and 64-127 (odd
head): their score matmuls auto-derive PE tile_position (0,0)/(64,0)
and run CONCURRENTLY in the top/bottom halves of the PE array (row
tiling).  Score psum alternates S,P,S,P,...,S between two tensors --
SC1 [128,1024] for single-group exps and SC2 [128,2048] for PAIRED
exps (one N=2048 activation amortizes the fixed ACT overhead; 5 pairs
+ 6 singles per block vs 16 singles) -- so consecutive activations
never read the same tensor and each group's psum write-after-read
clears two exps ahead.  Exp'd scores land in per-TRIPLET SBUF tiles
(tile-level RAW then resolves per-triplet, letting the last block's
AV start during its own exps).  AV for the previous block runs head
A in windows 1-4 / head B in 5-10 (chains early), norms at w7/w12,
and the O-projection is emitted post-block (2 tiles) + carried into
the next block's first windows (2), executing during the boundary
exps; pool-rotation-sensitive tiles are pre-allocated so they pair
with already-cast slots.  Startup: weights are host-pre-swizzled to
the SBUF layout (one 4KB-line DMA each), xT/yT interleave across the
sync+scalar hardware queues xT-first, and the K-projection's
psum->SBUF copies are split scalar/vector and partially deferred so
the first exp fires right after K's last matmul.  V-projection uses
four half-bank psum groups and spreads over blocks 0-1.
"""

import numpy as np
import ml_dtypes

import concourse.bass as bass
import concourse.bacc as bacc
import concourse.tile as tile
import concourse.mybir as mybir
from concourse.bass_utils import run_bass_kernel_spmd

BS, LX, LY, D, H, DK = 2, 2048, 2048, 1024, 16, 64
NCORES = 8
NGRP = 4             # head groups (tensor-parallel)
HD = H * DK // NGRP  # 256 head dims per core
NH = H // NGRP       # 4 heads per core
SXB = 512            # sx block
NSXB = LX // SXB     # 4
NSYT = LY // 128     # 16 sy tiles
NDC = D // 128       # 8 contraction chunks
NET = D // 128       # 8 output-feature tiles

F32 = mybir.dt.float32
BF16 = mybir.dt.bfloat16
EXPF = mybir.ActivationFunctionType.Exp

_compiled = {}


def _build(has_qk_bias: bool, has_mask: bool, n_cores: int = NCORES,
           with_collective: bool = False):
    nc = bacc.Bacc("TRN2", target_bir_lowering=False, debug=False,
                   num_devices=n_cores)

    xT = nc.dram_tensor("xT", [D, LX], BF16, kind="ExternalInput")
    yT = nc.dram_tensor("yT", [D, LY], BF16, kind="ExternalInput")
    # weights arrive pre-swizzled to the SBUF layout (row p = all
    # contraction chunks for partition p) so each is ONE DMA with 4KB
    # contiguous per-partition lines -- 512B-descriptor transfers run at
    # ~90 GB/s and were blocking the activation stream on their queue
    wq = nc.dram_tensor("wq", [128, NDC * HD], BF16, kind="ExternalInput")
    wk = nc.dram_tensor("wk", [128, NDC * HD], BF16, kind="ExternalInput")
    wv = nc.dram_tensor("wv", [128, NDC * HD], BF16, kind="ExternalInput")
    wo = nc.dram_tensor("wo", [128, 2 * D], BF16, kind="ExternalInput")
    if has_qk_bias:
        bq = nc.dram_tensor("bq", [HD], F32, kind="ExternalInput")
        bk = nc.dram_tensor("bk", [HD], F32, kind="ExternalInput")
    if has_mask:
        em = nc.dram_tensor("em", [LY, LX], BF16, kind="ExternalInput")
    # bf16 partials: halves the output DMA; the host accumulates the four
    # tensor-parallel partials in fp32
    out_ext = nc.dram_tensor("out", [D, LX], BF16, kind="ExternalOutput")

    stb = 2                       # ST2 ring (32KB/partition each)
    smb = 3 if has_mask else 4    # small-chain rings
    with tile.TileContext(nc) as tc:
        with (
            tc.tile_pool(name="persist", bufs=1) as pp,
            tc.tile_pool(name="st", bufs=stb) as stp,
            tc.tile_pool(name="ostage", bufs=3) as osp,
            tc.tile_pool(name="small", bufs=3) as smp,
            # scores region: 6 PSUM banks = 3 rotating groups of [128,1024]
            tc.tile_pool(name="scp", bufs=1, space="PSUM") as scp,
            # AV accumulators / V-proj / O-proj column tiles: 2 banks
            tc.tile_pool(name="avp", bufs=2, space="PSUM") as avp,
        ):
            # ---- static inputs -> SBUF --------------------------------
            # xT streams on the sync queue, yT concurrently on the vector
            # queue (HBM is the shared limit, but one queue alone tops out
            # ~280 GB/s on descriptor issue); wq/wk ride the scalar queue,
            # wv/wo the gpsimd queue so the projections are never
            # queue-gated.
            wq_sb = pp.tile([128, NDC * HD], BF16, tag="wq")
            wk_sb = pp.tile([128, NDC * HD], BF16, tag="wk")
            wv_sb = pp.tile([128, NDC * HD], BF16, tag="wv")
            xT_sb = pp.tile([128, NDC * LX], BF16, tag="xT")
            yT_sb = pp.tile([128, NDC * LY], BF16, tag="yT")
            wo_sb = pp.tile([128, 2 * D], BF16, tag="wo")
            # xT/yT interleave across both hardware DMA queues (sync and
            # scalar; one queue alone caps ~280 GB/s, two reach ~310+).
            # xT first so the Q projection drains the PE before K's
            # matmuls queue up; wq/wk lead the scalar queue, wv/wo ride
            # the (software-DGE) gpsimd queue since they're not urgent.
            nc.sync.dma_start(out=wq_sb[:], in_=wq[:, :])
            nc.sync.dma_start(out=wk_sb[:], in_=wk[:, :])
            for d in range(NDC):
                eng = nc.scalar if d % 2 == 0 else nc.sync
                eng.dma_start(out=xT_sb[:, d * LX:(d + 1) * LX],
                              in_=xT[d * 128:(d + 1) * 128, :])
            for d in range(NDC):
                eng = nc.scalar if d % 2 == 0 else nc.sync
                eng.dma_start(out=yT_sb[:, d * LY:(d + 1) * LY],
                              in_=yT[d * 128:(d + 1) * 128, :])
            nc.sync.dma_start(out=wv_sb[:], in_=wv[:, :])
            nc.gpsimd.dma_start(out=wo_sb[:], in_=wo[:, :])
            if has_qk_bias:
                bq_sb = pp.tile([128, 2], F32, tag="bq")
                bk_sb = pp.tile([128, 2], F32, tag="bk")
                nc.scalar.dma_start(out=bq_sb[:], in_=bq.ap().rearrange("(t p) -> p t", p=128))
                nc.scalar.dma_start(out=bk_sb[:], in_=bk.ap().rearrange("(t p) -> p t", p=128))

            ones_bf = pp.tile([1, 64], BF16, tag="ones")
            nc.vector.memset(ones_bf[:], 1.0)

            # scores psum: groups k%3 in {0,1} rotate through the halves
            # of SC2 (exp'd together, one N=2048 activation), k%3==2 goes
            # to SC1 (single N=1024 activation).  Two tensors so the
            # tile-level write-after-read dependencies coincide exactly
            # with the true pipeline hazards.
            SC2 = scp.tile([128, 2048], F32, tag="sc2")
            SC1 = scp.tile([128, 1024], F32, tag="sc1")

            def sc_group(k):
                # alternating S,P,S,P,...,S: singles (SC1) at k%3==0,
                # pairs (SC2) at k%3 in {1,2}.  Consecutive exps never
                # touch the same psum tensor, so the write-after-read for
                # each group resolves two exps ahead of its own exp.
                if k % 3 == 0:
                    return SC1[:]
                return SC2[:, (k % 3 - 1) * 1024:(k % 3) * 1024]

            # ---- Q^T / K^T projections: out [hd, seq] -----------------
            QT_sb = pp.tile([128, 2 * LX], BF16, tag="QT")
            KT_sb = pp.tile([128, 2 * LY], BF16, tag="KT")

            qk_parts = [(wq_sb, xT_sb, QT_sb, "bq"),
                        (wk_sb, yT_sb, KT_sb, "bk")]

            def emit_qk_proj_paced(part, av_tiles, defer_copies=False):
                # contraction-outer so each input chunk is consumed by 8
                # matmuls as soon as its DMA lands; 8 psum groups = 6 SC
                # half-slots + the 2 AV banks.  With defer_copies the
                # psum->SBUF copies are returned as closures so the first
                # score groups can start before all of K^T is copied out.
                w_sb, src_sb, dst_sb, bias_name = qk_parts[part]

                def group_ap(g):    # g = 0..7: t0 sb0-3, then t1 sb0-3
                    if g < 4:
                        return SC2[:, g * SXB:(g + 1) * SXB]
                    if g < 6:
                        return SC1[:, (g - 4) * SXB:(g - 3) * SXB]
                    return av_tiles[g - 6][:]
                for d in range(NDC):
                    for g in range(8):
                        t, sb = (0, g) if g < 4 else (1, g - 4)
                        nc.tensor.matmul(
                            group_ap(g),
                            lhsT=w_sb[:, d * HD + t * 128: d * HD + (t + 1) * 128],
                            rhs=src_sb[:, d * LX + sb * SXB: d * LX + sb * SXB + SXB],
                            start=(d == 0), stop=(d == NDC - 1),
                            skip_group_check=True)

                def mk_copy(g):
                    def f():
                        t, sb = (0, g) if g < 4 else (1, g - 4)
                        dst = dst_sb[:, t * LX + sb * SXB: t * LX + sb * SXB + SXB]
                        if has_qk_bias:
                            b_sb = bq_sb if bias_name == "bq" else bk_sb
                            nc.vector.tensor_scalar_add(dst, group_ap(g),
                                                        b_sb[:, t:t + 1])
                        elif defer_copies and g % 2 == 0:
                            # scalar engine is idle pre-first-exp; Copy is
                            # in every ACT table set (no reload)
                            nc.scalar.copy(dst, group_ap(g))
                        else:
                            nc.vector.tensor_copy(dst, group_ap(g))
                    return f
                copies = [mk_copy(g) for g in range(8)]
                if defer_copies:
                    return copies
                for f in copies:
                    f()

            # ---- V projection: out [seq, hd] interleaved with ones ----
            # V1 layout per sy tile: [128, NH*65] = 4 x (64 v-dims + ones)
            # (padded 64 cols so AV stationaries can be read 128 wide)
            V1_sb = pp.tile([128, NSYT * NH * 65 + 64], BF16, tag="V1")

            vp_ps = {}

            def emit_v_proj(st):
                # two [128, 256] psum half-bank groups per AV-pool tile:
                # 4 outstanding V-proj groups, so the matmuls never wait
                # on the psum->SBUF copy chain
                if st // 2 not in vp_ps:
                    vp_ps[st // 2] = avp.tile([128, SXB], F32, tag="av",
                                              name=f"vp{st}")
                ps = vp_ps[st // 2][:, (st % 2) * HD:(st % 2) * HD + HD]
                for d in range(NDC):
                    nc.tensor.matmul(
                        ps,
                        lhsT=yT_sb[:, d * LY + st * 128: d * LY + st * 128 + 128],
                        rhs=wv_sb[:, d * HD:(d + 1) * HD],
                        start=(d == 0), stop=(d == NDC - 1),
                        skip_group_check=True)
                dst = V1_sb[:, st * NH * 65:(st + 1) * NH * 65] \
                    .rearrange("p (h c) -> p h c", c=65)[:, :, 0:64]
                nc.vector.tensor_copy(dst, ps.rearrange("p (h c) -> p h c", c=64))

            # ---- mask (rare path): exp(mask)^T blocks per sx block ----
            em_blocks = {}

            def load_mask_block(sb):
                mb = stp.tile([128, NSYT * SXB], BF16, tag="mask", bufs=2)
                for st in range(NSYT):
                    nc.sync.dma_start(
                        out=mb[:, st * SXB:(st + 1) * SXB],
                        in_=em[st * 128:(st + 1) * 128, sb * SXB:(sb + 1) * SXB])
                em_blocks[sb] = mb

            # ---- attention: pair-blocks (sx block, head pair) ---------
            # pair-block p = (sb, hp): heads 2hp (partitions 0-63) and
            # 2hp+1 (partitions 64-127), both reading the ht=hp column
            # half of QT/KT.  16 score groups k=0..15: head k%2, sy tiles
            # (2*(k//2), 2*(k//2)+1), psum slot k%3.  ST2 columns follow
            # k, so the paired exp of groups (k, k+1) (k%3==0) writes one
            # contiguous [*, 2048] range.
            AO_sb = pp.tile([128, 2 * LX], BF16, tag="AO")
            pblocks = [(sb, hp) for sb in range(NSXB) for hp in range(2)]
            st_tiles = {}

            def emit_pair_scores(p, fillers):
                # fillers: list of 16 callables, one run after each score
                # group's matmuls + (possibly deferred) exp
                sb, hp = pblocks[p]
                if has_mask and hp == 0:
                    load_mask_block(sb)
                # exp'd scores: one tile per triplet of groups so the AV
                # matmuls' tile-level read deps resolve as soon as that
                # triplet's exps have run (not the whole block's)
                STt = [stp.tile([128, 3072 if t < 5 else 1024], BF16,
                                tag=f"st{t}", bufs=2, name=f"st{p}_{t}")
                       for t in range(6)]
                st_tiles[p] = STt

                def st_ap(kk, lo, hi):
                    return STt[kk // 3][:, (kk % 3) * 1024 + lo:
                                        (kk % 3) * 1024 + hi]
                for k in range(16):
                    hb, g = k % 2, k // 2
                    hr = hb * 64
                    ps = sc_group(k)
                    for j in range(2):
                        st = 2 * g + j
                        nc.tensor.matmul(
                            ps[:, j * SXB:(j + 1) * SXB],
                            lhsT=KT_sb[hr:hr + 64, hp * LY + st * 128: hp * LY + st * 128 + 128],
                            rhs=QT_sb[hr:hr + 64, hp * LX + sb * SXB: hp * LX + sb * SXB + SXB],
                            start=True, stop=True,
                            skip_group_check=True)
                    if k % 3 == 0:      # single exp from SC1
                        nc.scalar.activation(st_ap(k, 0, 1024), SC1[:],
                                             EXPF, scale=1.0 / (DK ** 0.5))
                    elif k % 3 == 2:    # paired exp over both SC2 halves
                        nc.scalar.activation(st_ap(k - 1, 0, 2048), SC2[:],
                                             EXPF, scale=1.0 / (DK ** 0.5))
                    if has_mask and k % 3 != 1:
                        mb = em_blocks[sb]
                        lo = k - 1 if (k % 3 == 2) else k
                        for kk in range(lo, k + 1):
                            gg = kk // 2
                            nc.vector.tensor_mul(
                                st_ap(kk, 0, 1024), st_ap(kk, 0, 1024),
                                mb[:, gg * 1024:(gg + 1) * 1024])
                    fillers[k]()

            # ---- AV + normalize chain ---------------------------------
            norm_state = {}
            norm_rr = {}

            def emit_av_chain(key, pav, last):
                dcp = smp.tile([1, SXB], F32, tag="den", bufs=2,
                               name=f"den{key[0]}_{key[1]}")
                nc.vector.tensor_copy(dcp[:], pav[64:65, :])
                rF = smp.tile([1, SXB], F32, tag="rf", bufs=2,
                              name=f"rf{key[0]}_{key[1]}")
                # ~18-bit accurate, 5x faster than InstReciprocal; must
                # read SBUF (custom-DVE op misreads PSUM on HW)
                nc.vector.reciprocal_approx_fast(rF[:], dcp[:])
                if last:
                    rrB = smp.tile([1, SXB], BF16, tag="rr", bufs=2,
                                   name=f"rr{key[0]}_{key[1]}")
                    nc.vector.tensor_copy(rrB[:], rF[:])
                    bc = None
                    norm_rr[key] = rrB
                else:
                    bc = smp.tile([64, SXB], F32, tag="bc", bufs=3,
                                  name=f"bc{key[0]}_{key[1]}")
                    nc.gpsimd.partition_broadcast(bc[:], rF[:])
                un = smp.tile([64, SXB], BF16, tag="un", bufs=3,
                              name=f"un{key[0]}_{key[1]}")
                nc.vector.tensor_copy(un[:], pav[0:64, :])
                norm_state[key] = (un, bc)

            def av_step(p, hb, st, pav, last):
                # one sy-tile step of AV accumulation for one head of
                # pair-block p; the chain rides the head's last step
                sb, hp = pblocks[p]
                h = 2 * hp + hb
                kk = 2 * (st // 2) + hb
                STt = st_tiles[p]
                src = STt[kk // 3][:, (kk % 3) * 1024 + (st % 2) * SXB:
                                   (kk % 3) * 1024 + (st % 2) * SXB + SXB]
                # stationary read 128 wide (overreads the neighbouring
                # head's data into psum rows 65-127, which nothing reads)
                # so the weight load qualifies for FWL and stays hidden
                nc.tensor.matmul(
                    pav[:],
                    lhsT=V1_sb[:, st * NH * 65 + h * 65:
                               st * NH * 65 + h * 65 + 128],
                    rhs=src,
                    start=(st == 0), stop=(st == NSYT - 1),
                    skip_group_check=True)
                if st == NSYT - 1:
                    emit_av_chain((p, hb), pav, last)

            def emit_norm_apply(p, hb):
                sb, hp = pblocks[p]
                h = 2 * hp + hb
                hr = hb * 64
                un, bc = norm_state.pop((p, hb))
                if bc is None:
                    pbc = avp.tile([128, SXB], F32, tag="av",
                                   name=f"pbc{p}_{hb}")
                    nc.tensor.matmul(pbc[0:64, :], lhsT=ones_bf[:],
                                     rhs=norm_rr.pop((p, hb))[:],
                                     start=True, stop=True)
                    bcap = pbc[0:64, :]
                else:
                    bcap = bc[:]
                nc.vector.tensor_mul(
                    AO_sb[hr:hr + 64,
                          hp * LX + sb * SXB: hp * LX + sb * SXB + SXB],
                    un[:], bcap)

            # ---- O-projection: et tiles in the AV banks ---------------
            ost4 = {}

            def emit_oproj_et(sb, et, tail=False, po=None):
                if po is None:
                    po = avp.tile([128, SXB], F32, tag="av",
                                  name=f"po{sb}_{et}")
                for c in range(2):
                    nc.tensor.matmul(
                        po[:],
                        lhsT=wo_sb[:, c * D + et * 128: c * D + (et + 1) * 128],
                        rhs=AO_sb[:, c * LX + sb * SXB: c * LX + sb * SXB + SXB],
                        start=(c == 0), stop=(c == 1),
                        skip_group_check=True)
                half = et // 4
                if (sb, half) not in ost4:
                    ost4[(sb, half)] = osp.tile([128, 4 * SXB], BF16,
                                                tag="ost", bufs=2,
                                                name=f"ost{sb}_{half}")
                dst = out_ext.ap().rearrange(
                    "(h e2 e p) (s c) -> h s p e2 e c", h=2, e2=2, e=2,
                    s=NSXB)
                ot = ost4[(sb, half)]
                q = et % 4
                if tail and et % 2 == 0:
                    # post-last-exp the scalar engine is idle: alternate
                    # the staging casts so DVE isn't the serial resource
                    nc.scalar.copy(ot[:, q * SXB:(q + 1) * SXB], po[:])
                else:
                    nc.vector.tensor_copy(ot[:, q * SXB:(q + 1) * SXB], po[:])
                last = (sb == NSXB - 1 and half == 1)
                if last and q == 1:
                    nc.sync.dma_start(
                        out=dst[half, sb][:, 0],
                        in_=ot[:, 0:2 * SXB].rearrange("p (e c) -> p e c",
                                                       e=2))
                elif q == 3:
                    del ost4[(sb, half)]
                    if last:
                        nc.sync.dma_start(
                            out=dst[half, sb][:, 1],
                            in_=ot[:, 2 * SXB:].rearrange("p (e c) -> p e c",
                                                          e=2))
                    else:
                        nc.sync.dma_start(
                            out=dst[half, sb].rearrange("p e2 e c -> p (e2 e) c"),
                            in_=ot[:].rearrange("p (e c) -> p e c", e=4))

            # ---- emission plan ----------------------------------------
            # pair-block 0: V projection rides the windows (AV banks are
            # idle); pair-block p>0: AV(p-1) front-loaded in windows
            # 0..7, chains at window 8-ish, norms at 11/12, pending
            # O-projection et tiles in the tail windows 12..15.
            av0 = avp.tile([128, SXB], F32, tag="av", name="qk0")
            av1 = avp.tile([128, SXB], F32, tag="av", name="qk1")
            emit_qk_proj_paced(0, (av0, av1))
            av2 = avp.tile([128, SXB], F32, tag="av", name="qk2")
            av3 = avp.tile([128, SXB], F32, tag="av", name="qk3")
            k_copies = emit_qk_proj_paced(1, (av2, av3), defer_copies=True)
            ones_cols = V1_sb[:, 0:NSYT * NH * 65].rearrange(
                "p (t h c) -> p t h c", t=NSYT, c=65)[:, :, :, 64:65]
            nc.vector.memset(ones_cols, 1.0)
            nc.vector.memset(V1_sb[:, NSYT * NH * 65:], 0.0)

            def noop():
                pass

            def combine(*fs):
                def f():
                    for g in fs:
                        g()
                return f

            # pair-block 0's first score groups reuse the SC psum the K
            # projection just filled: those six copies run up front (split
            # scalar/vector), only the two AV-bank copies are deferred.
            # V projection: one tile per window from window 2 on.
            # block 0 carries only 12 V-proj tiles (1/window from w3) --
            # 16 would oversubscribe the PE vs the exp budget; the last 4
            # ride block 1's first windows (AV(0) is delayed to w3+ there)
            for g in range(6):
                k_copies[g]()
            b0_fillers = [combine(*([k_copies[6], k_copies[7]]
                                    if w == 0 else []),
                                  *([lambda v=w - 3: emit_v_proj(v)]
                                    if 3 <= w < 15 else []))
                          for w in range(16)]
            emit_pair_scores(0, b0_fillers)

            # AV(p-1) steps per window of block p: head A (fast chain) in
            # windows 1-4, head B in 5-10; norms once each broadcast has
            # landed.  O-projection tiles are emitted AFTER the block's
            # last exp so they never sit between score matmuls in the PE
            # queue -- they execute during the boundary exps instead.
            # Block 1 runs everything ~2 windows later: its first windows
            # host block 0's last V-proj tiles.
            AV_A = [(), (0, 1, 2, 3), (4, 5, 6, 7), (8, 9, 10, 11),
                    (12, 13, 14, 15)] + [()] * 11
            AV_B = [()] * 5 + [(0, 1, 2), (3, 4, 5), (6, 7, 8),
                               (9, 10, 11), (12, 13), (14, 15)] + [()] * 5
            AV_A1 = [(), (), (), (0, 1, 2, 3), (4, 5, 6, 7), (8, 9, 10, 11),
                     (12, 13, 14, 15)] + [()] * 9
            AV_B1 = [()] * 7 + [(0, 1, 2), (3, 4, 5), (6, 7, 8),
                                (9, 10, 11), (12, 13), (14, 15)] + [()] * 3

            npb = len(pblocks)
            pending_oproj = []      # (sb, et) waiting for a slot
            carry_oproj = []        # ets deferred into the next block's
                                    # first windows (execute during its
                                    # first exps, after its k0 matmuls)
            for p in range(1, npb):
                # carry-over O-projection tiles allocate BEFORE the AV
                # accumulators so pool rotation pairs them with the
                # previous boundary's (already-cast) tiles
                carries = []
                for _ in range(min(2, len(carry_oproj))):
                    sb_, et_ = carry_oproj.pop(0)
                    pot = avp.tile([128, SXB], F32, tag="av",
                                   name=f"poc{sb_}_{et_}")
                    carries.append((sb_, et_, pot))
                if p == 1:
                    # pre-allocate the carried V-proj psum tiles so pool
                    # rotation pairs them with block 0's (already-copied)
                    # V-proj tiles rather than the live AV accumulators
                    vp_ps[6] = avp.tile([128, SXB], F32, tag="av",
                                        name="vp12")
                    vp_ps[7] = avp.tile([128, SXB], F32, tag="av",
                                        name="vp14")
                pavA = avp.tile([128, SXB], F32, tag="av", name=f"avA{p-1}")
                pavB = avp.tile([128, SXB], F32, tag="av", name=f"avB{p-1}")
                avA, avB = (AV_A1, AV_B1) if p == 1 else (AV_A, AV_B)
                fillers = []
                for w in range(16):
                    fs = []
                    if w < len(carries):
                        sb_, et_, pot = carries[w]
                        fs.append(lambda s=sb_, e=et_, t=pot:
                                  emit_oproj_et(s, e, po=t))
                    if p == 1 and w < 4:
                        fs.append(lambda v=12 + w: emit_v_proj(v))
                    fs += [(lambda s=s: av_step(p - 1, 0, s, pavA, False))
                           for s in avA[w]]
                    fs += [(lambda s=s: av_step(p - 1, 1, s, pavB, False))
                           for s in avB[w]]
                    fillers.append(combine(*fs))
                # norms for p-1 once the gpsimd broadcast has landed
                nwA, nwB = (9, 14) if p == 1 else (7, 12)
                fillers[nwA] = combine(fillers[nwA],
                                       lambda p_=p - 1: emit_norm_apply(p_, 0))
                fillers[nwB] = combine(fillers[nwB],
                                       lambda p_=p - 1: emit_norm_apply(p_, 1))
                if p == npb - 1:
                    # pre-emit the last block's own AV for already-exp'd
                    # triplets into its tail windows
                    pav7A = avp.tile([128, SXB], F32, tag="av", name="avAL")
                    pav7B = avp.tile([128, SXB], F32, tag="av", name="avBL")
                    pre = [(0, s) for s in range(9)] + [(1, s) for s in range(9)]
                    for w in range(10, 16):
                        take, pre = pre[:3], pre[3:]
                        fillers[w] = combine(fillers[w], *[
                            (lambda hb=hb, s=s: av_step(
                                npb - 1, hb, s, pav7A if hb == 0 else pav7B,
                                False))
                            for hb, s in take])
                emit_pair_scores(p, fillers)
                # O-projection of the sx block whose norms were applied
                # during THIS block's windows (emitted post-block, so the
                # matmuls execute during the boundary exps)
                osb, hpdone = pblocks[p - 1]
                if hpdone == 1:
                    pending_oproj.extend((osb, et) for et in range(NET))
                if p < npb - 1:
                    for _ in range(2):
                        if pending_oproj:
                            sb_, et_ = pending_oproj.pop(0)
                            emit_oproj_et(sb_, et_)
                    for _ in range(2):
                        if pending_oproj:
                            carry_oproj.append(pending_oproj.pop(0))

            # tail: remaining AV steps of the last pair-block; leftover
            # O-projection tiles fill the PE while the final normalize
            # chains run on DVE/gpsimd; then the last sx block's
            # O-projection
            for st in range(9, NSYT):
                av_step(npb - 1, 0, st, pav7A, False)
                av_step(npb - 1, 1, st, pav7B, st == NSYT - 1)
            while pending_oproj:
                sb_, et_ = pending_oproj.pop(0)
                emit_oproj_et(sb_, et_, tail=True)
            emit_norm_apply(npb - 1, 0)
            emit_norm_apply(npb - 1, 1)
            for et in range(NET):
                emit_oproj_et(NSXB - 1, et, tail=True)

    nc.compile()
    return nc


def _get_compiled(has_qk_bias: bool, has_mask: bool):
    key = (has_qk_bias, has_mask)
    if key not in _compiled:
        _compiled[key] = _build(has_qk_bias, has_mask)
    return _compiled[key]


def _prep_inputs(x, y, mask, Wq, bq, Wk, bk, Wv, bv, Wo, bo,
                 has_qk_bias, has_mask):
    bf = ml_dtypes.bfloat16
    xT = [np.ascontiguousarray(x[b].T).astype(bf) for b in range(BS)]
    yT = [np.ascontiguousarray(y[b].T).astype(bf) for b in range(BS)]
    if has_mask:
        em = [np.ascontiguousarray(np.exp(mask[b, 0]).T).astype(bf)
              for b in range(BS)]
    def swz(W):
        # [n*128, m] -> [128, n*m]: row p holds all contraction chunks
        # for partition p (matches the SBUF tile layout; 4KB DMA lines)
        n = W.shape[0] // 128
        return np.ascontiguousarray(
            W.reshape(n, 128, -1).transpose(1, 0, 2).reshape(128, -1)
        ).astype(bf)

    in_maps = []
    for c in range(NCORES):
        b, g = c // NGRP, c % NGRP
        sl = slice(g * HD, (g + 1) * HD)
        m = {
            "xT": xT[b], "yT": yT[b],
            "wq": swz(Wq[:, sl]),
            "wk": swz(Wk[:, sl]),
            "wv": swz(Wv[:, sl]),
            "wo": swz(Wo[sl, :]),
        }
        if has_qk_bias:
            m["bq"] = np.ascontiguousarray(bq[sl]).astype(np.float32)
            m["bk"] = np.ascontiguousarray(bk[sl]).astype(np.float32)
        if has_mask:
            m["em"] = em[b]
        in_maps.append(m)
    return in_maps


def kernel(x, y, mask, Wq, bq, Wk, bk, Wv, bv, Wo, bo):
    x = np.asarray(x, np.float32)
    y = np.asarray(y, np.float32)
    mask = np.asarray(mask, np.float32)
    has_qk_bias = bool(np.any(bq) or np.any(bk))
    has_mask = bool(np.any(mask))
    nc = _get_compiled(has_qk_bias, has_mask)
    in_maps = _prep_inputs(x, y, mask, Wq, bq, Wk, bk, Wv, bv, Wo, bo,
                           has_qk_bias, has_mask)
    res = run_bass_kernel_spmd(nc, in_maps, list(range(NCORES)))
    out = np.empty((BS, LX, D), np.float32)
    for b in range(BS):
        OT = res.results[b * NGRP]["out"].astype(np.float32)
        for r in range(1, NGRP):
            OT += res.results[b * NGRP + r]["out"].astype(np.float32)
        out[b] = OT.T
    bv = np.asarray(bv, np.float32)
    bo = np.asarray(bo, np.float32)
    if bv.any() or bo.any():
        # softmax rows sum to 1 => v-bias passes through attention exactly
        out += (bv @ np.asarray(Wo, np.float32) + bo)[None, None, :]
    return out
